# revision 1
# baseline (speedup 1.0000x reference)
"""Trainium2 Bass kernel for nn_CrossAttention (B=4, N=M=1024, C=768, H=12, D=64).

Sharding: pure data-parallel over 8 cores. Core c handles batch b = c // 2 and
query rows [512*(c%2), 512*(c%2)+512). Each core computes K/V for its batch
(duplicated across the 2 cores sharing a batch) so no collectives are needed.

All host-side preprocessing (sharding + transposes) is done in numpy so the
device kernel performs zero layout transposes:
  - xT  [768, 512]   = x[b, n0:n0+512, :].T        (c-major for Q projection)
  - yT  [768, 1024]  = y[b].T                      (c-major for K/V projection)
  - wqT/wkT/wvT/wpT [768, 768] = W.T               (c-major weights)
  - yw  [1, 1024], bp [1, 768]

Device dataflow (all matmuls in float32r: full PE rate at fp32 storage):
  QT[co,n]  = sum_c wqT[c,co] xT[c,n]              (feature-major Q)
  KT[co,m]  = sum_c wkT[c,co] yT[c,m] + yw[m]      (bias via rank-1 ones matmul)
  V[m,cv]   = sum_c yT[c,m] wvT[c,cv]              (sequence-major V, stored with a
                                                    ones column per head: 65-col strides)
  per head h, per m-chunk: ST[m,n] = KT_h[:,m]^T @ QT_h  -> exp(SCALE*ST) on ACT
  O'[d+1,n] accum += V_h[m, d|1]^T @ expST[m,n]    (row 64 = softmax denominator Z)
  OT[d,n]   = O'[0:64] * (1/Z) broadcast           (rank-1 ones matmul broadcast)
  out[n,co] = sum_ci OT[ci,n] wpT[ci,co] + bp[co]  (bias via rank-1 ones matmul)
"""

import sys

for _p in ("/opt/trn_rl_repo",):
    if _p not in sys.path:
        sys.path.insert(0, _p)

import numpy as np
from contextlib import ExitStack

import concourse.bass as bass
import concourse.mybir as mybir
import concourse.tile as tile
from concourse import bacc

F32 = mybir.dt.float32
F32R = mybir.dt.float32r

B = 4
N = 1024
M = 1024
C = 768
H = 12
D = 64
NSH = 512            # query rows per core
CK = C // 128        # 6 chunks of the feature dim
MK = M // 128        # 8 chunks of the key dim
SCALE = D ** -0.5
N_CORES = 8


def build_bass():
    nc = bacc.Bacc("TRN2", target_bir_lowering=False, debug=False)

    xT = nc.dram_tensor("xT", [C, NSH], F32R, kind="ExternalInput").ap()
    yT = nc.dram_tensor("yT", [C, M], F32R, kind="ExternalInput").ap()
    yw = nc.dram_tensor("yw", [1, M], F32R, kind="ExternalInput").ap()
    wqT = nc.dram_tensor("wqT", [C, C], F32R, kind="ExternalInput").ap()
    wkT = nc.dram_tensor("wkT", [C, C], F32R, kind="ExternalInput").ap()
    wvT = nc.dram_tensor("wvT", [C, C], F32R, kind="ExternalInput").ap()
    wpT = nc.dram_tensor("wpT", [C, C], F32R, kind="ExternalInput").ap()
    bp = nc.dram_tensor("bp", [1, C], F32R, kind="ExternalInput").ap()
    ones_in = nc.dram_tensor("ones_in", [1, 128], F32R, kind="ExternalInput").ap()
    out = nc.dram_tensor("out", [NSH, C], F32, kind="ExternalOutput").ap()

    wqT_c = wqT.rearrange("(k p) n -> k p n", p=128)
    wkT_c = wkT.rearrange("(k p) n -> k p n", p=128)
    wvT_c = wvT.rearrange("(k p) n -> k p n", p=128)
    wpT_c = wpT.rearrange("(k p) n -> k p n", p=128)
    xT_c = xT.rearrange("(k p) n -> k p n", p=128)
    yT_c = yT.rearrange("(k p) n -> k p n", p=128)
    out_c = out.rearrange("(k p) n -> k p n", p=128)

    with tile.TileContext(nc) as tc, ExitStack() as ctx:
        wpool = ctx.enter_context(tc.tile_pool(name="w", bufs=18))
        xpool = ctx.enter_context(tc.tile_pool(name="xt", bufs=CK))
        ypool = ctx.enter_context(tc.tile_pool(name="yt", bufs=CK))
        qpool = ctx.enter_context(tc.tile_pool(name="qt", bufs=CK))
        kpool = ctx.enter_context(tc.tile_pool(name="kt", bufs=CK))
        vpool = ctx.enter_context(tc.tile_pool(name="vs", bufs=MK))
        opool = ctx.enter_context(tc.tile_pool(name="ot", bufs=CK))
        epool = ctx.enter_context(tc.tile_pool(name="es", bufs=6))
        outpool = ctx.enter_context(tc.tile_pool(name="outs", bufs=2))
        cpool = ctx.enter_context(tc.tile_pool(name="const", bufs=1))
        zpool = ctx.enter_context(tc.tile_pool(name="z", bufs=2))
        ppool = ctx.enter_context(tc.tile_pool(name="pp", bufs=2, space="PSUM"))
        spool = ctx.enter_context(tc.tile_pool(name="sp", bufs=3, space="PSUM"))
        opsum = ctx.enter_context(tc.tile_pool(name="op", bufs=2, space="PSUM"))
        zbpool = ctx.enter_context(tc.tile_pool(name="zb", bufs=1, space="PSUM"))

        # ---- constants / small inputs ----
        ones = cpool.tile([1, 128], F32R, tag="ones")
        nc.sync.dma_start(out=ones, in_=ones_in)
        yw_s = cpool.tile([1, M], F32R, tag="yws")
        nc.sync.dma_start(out=yw_s, in_=yw)
        bp_s = cpool.tile([1, C], F32R, tag="bps")
        nc.sync.dma_start(out=bp_s, in_=bp)

        # ---- weight / activation loads (emission order = priority order) ----
        wq, xt, wk, yt, wv, wp = [], [], [], [], [], []
        # Interleave loads and split across both HWDGE issuers (SP + ACT)
        # so the K/V projections are not DMA-starved: yT/wk/wv arrive while
        # the Q projection computes.
        for i in range(CK):
            t = wpool.tile([128, C], F32R, tag="w", name=f"wq{i}")
            nc.sync.dma_start(out=t, in_=wqT_c[i])
            wq.append(t)
            t = xpool.tile([128, NSH], F32R, tag="xt", name=f"xt{i}")
            nc.sync.dma_start(out=t, in_=xT_c[i])
            xt.append(t)
            t = ypool.tile([128, M], F32R, tag="yt", name=f"yt{i}")
            nc.scalar.dma_start(out=t, in_=yT_c[i])
            yt.append(t)
            t = wpool.tile([128, C], F32R, tag="w", name=f"wk{i}")
            nc.scalar.dma_start(out=t, in_=wkT_c[i])
            wk.append(t)
        for i in range(CK):
            t = wpool.tile([128, C], F32R, tag="w", name=f"wv{i}")
            nc.scalar.dma_start(out=t, in_=wvT_c[i])
            wv.append(t)
        for i in range(CK):
            t = wpool.tile([128, C], F32R, tag="w", name=f"wp{i}")  # reuses wq slots
            nc.sync.dma_start(out=t, in_=wpT_c[i])
            wp.append(t)

        # ---- Q projection: QT[co*128:.., n] ----
        qt = []
        for co in range(CK):
            ps = ppool.tile([128, 512], F32, tag="pp")
            for ci in range(CK):
                nc.tensor.matmul(
                    ps,
                    wq[ci][:, co * 128:(co + 1) * 128],
                    xt[ci],
                    start=(ci == 0),
                    stop=(ci == CK - 1),
                )
            t = qpool.tile([128, NSH], F32R, tag="qt")
            nc.vector.tensor_copy(t, ps)
            qt.append(t)

        # ---- K projection + yw bias: KT[co*128:.., m] ----
        kt = []
        for co in range(CK):
            t = kpool.tile([128, M], F32R, tag="kt")
            kt.append(t)
            for mh in range(2):
                ps = ppool.tile([128, 512], F32, tag="pp")
                for ci in range(CK):
                    nc.tensor.matmul(
                        ps,
                        wk[ci][:, co * 128:(co + 1) * 128],
                        yt[ci][:, mh * 512:(mh + 1) * 512],
                        start=(ci == 0),
                        stop=False,
                    )
                # += ones^T @ yw  (broadcast yw over the 128 co-partitions)
                nc.tensor.matmul(
                    ps,
                    ones,
                    yw_s[:, mh * 512:(mh + 1) * 512],
                    start=False,
                    stop=True,
                )
                nc.vector.tensor_copy(t[:, mh * 512:(mh + 1) * 512], ps)

        # ---- V projection (sequence-major, 65-col per-head layout) ----
        vt = []
        for mc in range(MK):
            t = vpool.tile([128, 65 * H], F32R, tag="vs")
            vt.append(t)
            t3 = t.rearrange("p (h e) -> p h e", e=65)
            # ones column per head via broadcast DMA (memset can't write f32r)
            ones_bcast = bass.AP(
                tensor=ones_in.tensor,
                offset=0,
                ap=[[0, 128], [0, H], [1, 1]],
            )
            nc.sync.dma_start(out=t3[:, :, 64:65], in_=ones_bcast)
            for nh in range(2):
                ps = ppool.tile([128, 384], F32, tag="pp")
                for ci in range(CK):
                    nc.tensor.matmul(
                        ps,
                        yt[ci][:, mc * 128:(mc + 1) * 128],
                        wv[ci][:, nh * 384:(nh + 1) * 384],
                        start=(ci == 0),
                        stop=(ci == CK - 1),
                    )
                src = ps.rearrange("p (h e) -> p h e", e=64)
                dst = t3[:, nh * 6:(nh + 1) * 6, 0:64]
                nc.vector.tensor_copy(dst, src)

        # ---- attention (streaming over m-chunks; no max subtraction) ----
        ot = [
            opool.tile([128, NSH], F32R, tag="ot", name=f"ot{i}")
            for i in range(CK)
        ]
        for h in range(H):
            ktile = kt[h // 2]
            qtile = qt[h // 2]
            pr = 64 * (h % 2)
            op = opsum.tile([128, 512], F32, tag="op")
            for mc in range(MK):
                sp = spool.tile([128, 512], F32, tag="sp")
                nc.tensor.matmul(
                    sp,
                    ktile[pr:pr + 64, mc * 128:(mc + 1) * 128],
                    qtile[pr:pr + 64, :],
                    start=True,
                    stop=True,
                )
                es = epool.tile([128, 512], F32R, tag="es")
                nc.scalar.activation(
                    es, sp, mybir.ActivationFunctionType.Exp, scale=SCALE
                )
                nc.tensor.matmul(
                    op[0:65, :],
                    vt[mc][:, 65 * h:65 * h + 65],
                    es,
                    start=(mc == 0),
                    stop=(mc == MK - 1),
                )
            zi = zpool.tile([1, 512], F32R, tag="z")
            with nc.allow_low_precision(reason="f32r is fp32-width storage"):
                nc.vector.reciprocal(zi, op[64:65, :])
            zb = zbpool.tile([64, 512], F32, tag="zb")
            nc.tensor.matmul(zb, ones[:, 0:64], zi, start=True, stop=True)
            zbs = zpool.tile([64, 512], F32, tag="zbs")
            nc.vector.tensor_copy(zbs, zb)
            nc.vector.tensor_tensor(
                ot[h // 2][pr:pr + 64, :], op[0:64, :], zbs, mybir.AluOpType.mult
            )

        # ---- output projection + bias ----
        for n4 in range(4):
            outs = outpool.tile([128, C], F32, tag="outs")
            for nh in range(2):
                ps = ppool.tile([128, 384], F32, tag="pp")
                for ci in range(CK):
                    nc.tensor.matmul(
                        ps,
                        ot[ci][:, n4 * 128:(n4 + 1) * 128],
                        wp[ci][:, nh * 384:(nh + 1) * 384],
                        start=(ci == 0),
                        stop=False,
                    )
                nc.tensor.matmul(
                    ps,
                    ones,
                    bp_s[:, nh * 384:(nh + 1) * 384],
                    start=False,
                    stop=True,
                )
                nc.vector.tensor_copy(outs[:, nh * 384:(nh + 1) * 384], ps)
            nc.sync.dma_start(out=out_c[n4], in_=outs)

    if not nc.is_finalized():
        nc.finalize()
    return nc


_NC_CACHE = None


def _get_nc():
    global _NC_CACHE
    if _NC_CACHE is None:
        _NC_CACHE = build_bass()
    return _NC_CACHE


def _round_f32r(a):
    """Round to the bf16+bf16 representable set the PE's fp32r path uses."""
    import ml_dtypes

    a32 = np.asarray(a, np.float32)
    hi = a32.astype(ml_dtypes.bfloat16).astype(np.float32)
    lo = (a32 - hi).astype(ml_dtypes.bfloat16).astype(np.float32)
    return hi + lo


def make_in_maps(x, y, yw, Wq, Wk, Wv, Wp, bp):
    x = _round_f32r(np.asarray(x, dtype=np.float32))
    y = _round_f32r(np.asarray(y, dtype=np.float32))
    yw = _round_f32r(np.asarray(yw, dtype=np.float32))
    wqT = _round_f32r(np.ascontiguousarray(np.asarray(Wq, dtype=np.float32).T))
    wkT = _round_f32r(np.ascontiguousarray(np.asarray(Wk, dtype=np.float32).T))
    wvT = _round_f32r(np.ascontiguousarray(np.asarray(Wv, dtype=np.float32).T))
    wpT = _round_f32r(np.ascontiguousarray(np.asarray(Wp, dtype=np.float32).T))
    bp = np.asarray(bp, dtype=np.float32).reshape(1, C)

    in_maps = []
    for c in range(N_CORES):
        b, half = divmod(c, 2)
        n0 = half * NSH
        in_maps.append(
            {
                "xT": np.ascontiguousarray(x[b, n0:n0 + NSH, :].T),
                "yT": np.ascontiguousarray(y[b].T),
                "yw": np.ascontiguousarray(yw[b].reshape(1, M)),
                "wqT": wqT,
                "wkT": wkT,
                "wvT": wvT,
                "wpT": wpT,
                "bp": bp,
                "ones_in": np.ones((1, 128), np.float32),
            }
        )
    return in_maps


def run(inputs, trace=False):
    """Returns (full_output, BassKernelResults)."""
    from concourse.bass_utils import run_bass_kernel_spmd

    nc = _get_nc()
    in_maps = make_in_maps(**inputs)
    res = run_bass_kernel_spmd(
        nc, in_maps, list(range(N_CORES)), trace=trace
    )
    full = np.empty((B, N, C), dtype=np.float32)
    for c in range(N_CORES):
        b, half = divmod(c, 2)
        n0 = half * NSH
        full[b, n0:n0 + NSH, :] = res.results[c]["out"]
    return full, res


def kernel(**inputs):
    full, _ = run(inputs, trace=False)
    return full



# revision 8
# speedup vs baseline: 1.7772x; 1.7772x over previous
"""Trainium2 Bass kernel for nn_CrossAttention (B=4, N=M=1024, C=768, H=12, D=64).

Sharding: pure data-parallel over 8 cores. Core c handles batch b = c // 2 and
query rows [512*(c%2), 512*(c%2)+512). Each core computes K/V for its batch
(duplicated across the 2 cores sharing a batch) so no collectives are needed.

v2: all-bf16 datapath (fp32 PSUM accumulation). Rationale from the v1 trace:
the fp32r moving stream runs at ~2 cycles/row and trips the power throttler
(HAM K=4/8 for 249 of 342 us), while bf16 streams at 1 cycle/row. Layout:
  xT  [768, 512]   = x[b, n0:n0+512, :].T   (c-major for Q projection)
  yT  [768, 1024]  = y[b].T                 (c-major for K/V projection)
  w*T [768, 768]   = W.T                    (c-major weights), yw/bp fp32 rows

Device dataflow (all matmuls bf16 x bf16 -> fp32 PSUM):
  QT[co,n] = sum_c wqT[c,co] xT[c,n]        per head-pair co, just-in-time
  KT[co,m] = sum_c wkT[c,co] yT[c,m]; + yw via DVE add with a DMA-replicated
             [128,1024] yw row (no bias matmuls)
  V[m,cv]  = sum_c yT[c,m] wvT[c,cv]        65-col per-head layout, ones col
             via gpsimd memset; emitted inside head-pair 0's chunk loop
  attention per head PAIR (one KT/QT tile): for each m-chunk, two K=64
  S-matmuls at array tile positions (0,0)/(64,0) into one [128,1024] PSUM
  tile, ONE exp over [128,1024] -> bf16, two PV matmuls accumulating
  [128,512] where the V tile's cols 64:128 are all ones, so PSUM rows
  64:128 hold Z replicated 64x. 1/Z via reciprocal_approx_fast on those
  rows directly (no partition broadcast needed), DVE multiply.
  out[n,co] = sum_ci OT[ci,n] wpT[ci,co]; + bp via DVE add (replicated row).
"""

import sys

for _p in ("/opt/trn_rl_repo",):
    if _p not in sys.path:
        sys.path.insert(0, _p)

import numpy as np
from contextlib import ExitStack

import concourse.bass as bass
import concourse.mybir as mybir
import concourse.tile as tile
from concourse import bacc

F32 = mybir.dt.float32
BF16 = mybir.dt.bfloat16

B = 4
N = 1024
M = 1024
C = 768
H = 12
D = 64
NSH = 512            # query rows per core
CK = C // 128        # 6 chunks of the feature dim
MK = M // 128        # 8 chunks of the key dim
HP = H // 2          # 6 head pairs (one KT/QT co-chunk each)
SCALE = D ** -0.5
N_CORES = 8


def build_bass():
    nc = bacc.Bacc("TRN2", target_bir_lowering=False, debug=False)

    xT = nc.dram_tensor("xT", [C, NSH], BF16, kind="ExternalInput").ap()
    yT = nc.dram_tensor("yT", [C, M], BF16, kind="ExternalInput").ap()
    ywf = nc.dram_tensor("ywf", [1, M], F32, kind="ExternalInput").ap()
    wqT = nc.dram_tensor("wqT", [C, C], BF16, kind="ExternalInput").ap()
    wkT = nc.dram_tensor("wkT", [C, C], BF16, kind="ExternalInput").ap()
    wvT = nc.dram_tensor("wvT", [C, C], BF16, kind="ExternalInput").ap()
    wpT = nc.dram_tensor("wpT", [C, C], BF16, kind="ExternalInput").ap()
    bpf = nc.dram_tensor("bpf", [1, C], F32, kind="ExternalInput").ap()
    out = nc.dram_tensor("out", [NSH, C], F32, kind="ExternalOutput").ap()

    # p-major views so each full tensor loads in ONE large DMA
    wq_r = wqT.rearrange("(k p) n -> p k n", p=128)
    wk_r = wkT.rearrange("(k p) n -> p k n", p=128)
    wv_r = wvT.rearrange("(k p) n -> p k n", p=128)
    wp_r = wpT.rearrange("(k p) n -> p k n", p=128)
    xT_r = xT.rearrange("(k p) n -> p k n", p=128)
    yT_r = yT.rearrange("(k p) n -> p k n", p=128)
    out_c = out.rearrange("(k p) n -> k p n", p=128)

    with tile.TileContext(nc) as tc, ExitStack() as ctx:
        wpool = ctx.enter_context(tc.tile_pool(name="w", bufs=4))
        cpool = ctx.enter_context(tc.tile_pool(name="const", bufs=1))
        qpool = ctx.enter_context(tc.tile_pool(name="qt", bufs=3))
        kpool = ctx.enter_context(tc.tile_pool(name="kt", bufs=3))
        vpool = ctx.enter_context(tc.tile_pool(name="vs", bufs=MK))
        opool = ctx.enter_context(tc.tile_pool(name="ot", bufs=CK))
        epool = ctx.enter_context(tc.tile_pool(name="es", bufs=3))
        outpool = ctx.enter_context(tc.tile_pool(name="outs", bufs=2))
        zpool = ctx.enter_context(tc.tile_pool(name="z", bufs=4))
        ppool = ctx.enter_context(tc.tile_pool(name="pp", bufs=2, space="PSUM"))
        oppool = ctx.enter_context(tc.tile_pool(name="op", bufs=4, space="PSUM"))

        # ---- input loads; order matters (FIFO per HWDGE ring) ----
        # scalar (qActDynamicHW) ring: wq first (Q proj stationary), yT, wp
        # sync   (qSPDynamicHW)  ring: xT, yw row, wk, wv, bp row
        wq = wpool.tile([128, CK, C], BF16, tag="w", name="wq")
        nc.scalar.dma_start(out=wq, in_=wq_r)
        xt = cpool.tile([128, CK, NSH], BF16, tag="xt")
        nc.sync.dma_start(out=xt, in_=xT_r)
        ywb = cpool.tile([128, M], F32, tag="ywb")
        nc.sync.dma_start(
            out=ywb,
            in_=bass.AP(tensor=ywf.tensor, offset=0, ap=[[0, 128], [1, M]]),
        )
        yt = cpool.tile([128, CK, M], BF16, tag="yt")
        nc.scalar.dma_start(out=yt, in_=yT_r)
        wk = wpool.tile([128, CK, C], BF16, tag="w", name="wk")
        nc.sync.dma_start(out=wk, in_=wk_r)
        wv = wpool.tile([128, CK, C], BF16, tag="w", name="wv")
        nc.sync.dma_start(out=wv, in_=wv_r)
        wp = wpool.tile([128, CK, C], BF16, tag="w", name="wp")
        nc.scalar.dma_start(out=wp, in_=wp_r)
        bpb = cpool.tile([128, C], F32, tag="bpb")
        nc.sync.dma_start(
            out=bpb,
            in_=bass.AP(tensor=bpf.tensor, offset=0, ap=[[0, 128], [1, C]]),
        )
        # preload the ACT exp table off the critical path
        warm = cpool.tile([1, 8], F32, tag="warm")
        nc.scalar.activation(
            warm, ywb[0:1, 0:8], mybir.ActivationFunctionType.Exp, scale=SCALE
        )

        def qproj(co):
            ps = ppool.tile([128, 1024], F32, tag="pp")
            for ci in range(CK):
                nc.tensor.matmul(
                    ps[:, 0:512],
                    wq[:, ci, co * 128:(co + 1) * 128],
                    xt[:, ci, :],
                    start=(ci == 0),
                    stop=(ci == CK - 1),
                )
            t = qpool.tile([128, NSH], BF16, tag="qt")
            nc.vector.tensor_copy(t, ps[:, 0:512])
            return t

        def kproj(co):
            t = kpool.tile([128, M], BF16, tag="kt")
            ps = ppool.tile([128, 1024], F32, tag="pp")
            for mh in range(2):
                sl = slice(mh * 512, (mh + 1) * 512)
                for ci in range(CK):
                    nc.tensor.matmul(
                        ps[:, sl],
                        wk[:, ci, co * 128:(co + 1) * 128],
                        yt[:, ci, sl],
                        start=(ci == 0),
                        stop=(ci == CK - 1),
                    )
            for mh in range(2):
                sl = slice(mh * 512, (mh + 1) * 512)
                nc.vector.tensor_tensor(
                    t[:, sl], ps[:, sl], ywb[:, sl], mybir.AluOpType.add
                )
            return t

        def vproj(mc):
            t = vpool.tile([128, H, 128], BF16, tag="vs")
            nc.gpsimd.memset(t[:, :, 64:128], 1.0)
            ps = ppool.tile([128, 1024], F32, tag="pp")
            for nh in range(2):
                sl = slice(nh * 512, nh * 512 + 384)
                for ci in range(CK):
                    nc.tensor.matmul(
                        ps[:, sl],
                        yt[:, ci, mc * 128:(mc + 1) * 128],
                        wv[:, ci, nh * 384:(nh + 1) * 384],
                        start=(ci == 0),
                        stop=(ci == CK - 1),
                    )
            for nh in range(2):
                src = ps[:, nh * 512:nh * 512 + 384].rearrange(
                    "p (h e) -> p h e", e=64
                )
                nc.vector.tensor_copy(t[:, nh * 6:(nh + 1) * 6, 0:64], src)
            return t

        vt = [None] * MK
        ot = [None] * HP

        def attn_pair(hp, qtile, ktile, build_v):
            h0, h1 = 2 * hp, 2 * hp + 1
            op0 = oppool.tile([128, 512], F32, tag="op", name=f"op{h0}")
            op1 = oppool.tile([128, 512], F32, tag="op", name=f"op{h1}")
            for mc in range(MK):
                if build_v:
                    vt[mc] = vproj(mc)
                sp = ppool.tile([128, 1024], F32, tag="pp")
                # two K=64 matmuls at PE array tile positions (0,0)/(64,0);
                # disjoint sub-arrays + different PSUM banks -> overlap
                nc.tensor.matmul(
                    sp[:, 0:512],
                    ktile[0:64, mc * 128:(mc + 1) * 128],
                    qtile[0:64, :],
                    start=True,
                    stop=True,
                )
                nc.tensor.matmul(
                    sp[:, 512:1024],
                    ktile[64:128, mc * 128:(mc + 1) * 128],
                    qtile[64:128, :],
                    start=True,
                    stop=True,
                )
                es = epool.tile([128, 1024], BF16, tag="es")
                nc.scalar.activation(
                    es, sp, mybir.ActivationFunctionType.Exp, scale=SCALE
                )
                nc.tensor.matmul(
                    op0, vt[mc][:, h0, :], es[:, 0:512],
                    start=(mc == 0), stop=(mc == MK - 1),
                )
                nc.tensor.matmul(
                    op1, vt[mc][:, h1, :], es[:, 512:1024],
                    start=(mc == 0), stop=(mc == MK - 1),
                )
            t = opool.tile([128, NSH], BF16, tag="ot", name=f"ot{hp}")
            for j, op in enumerate((op0, op1)):
                zr = zpool.tile([64, 512], F32, tag="z")
                # reciprocal_approx_fast (custom DVE op) returns garbage on
                # this hardware; the stock iterative-divide op works.
                nc.vector.reciprocal(zr, op[64:128, :])
                nc.vector.tensor_tensor(
                    t[j * 64:(j + 1) * 64, :], op[0:64, :], zr,
                    mybir.AluOpType.mult,
                )
            return t

        # head pair 0 with V projection interleaved into its chunk loop
        q0 = qproj(0)
        k0 = kproj(0)
        ot[0] = attn_pair(0, q0, k0, build_v=True)
        for hp in range(1, HP):
            qt_ = qproj(hp)
            kt_ = kproj(hp)
            ot[hp] = attn_pair(hp, qt_, kt_, build_v=False)

        # ---- output projection + bias ----
        for n4 in range(4):
            ps = ppool.tile([128, 1024], F32, tag="pp")
            for nh in range(2):
                sl = slice(nh * 512, nh * 512 + 384)
                for ci in range(CK):
                    nc.tensor.matmul(
                        ps[:, sl],
                        ot[ci][:, n4 * 128:(n4 + 1) * 128],
                        wp[:, ci, nh * 384:(nh + 1) * 384],
                        start=(ci == 0),
                        stop=(ci == CK - 1),
                    )
            outs = outpool.tile([128, C], F32, tag="outs")
            for nh in range(2):
                nc.vector.tensor_tensor(
                    outs[:, nh * 384:(nh + 1) * 384],
                    ps[:, nh * 512:nh * 512 + 384],
                    bpb[:, nh * 384:(nh + 1) * 384],
                    mybir.AluOpType.add,
                )
            nc.sync.dma_start(out=out_c[n4], in_=outs)

    if not nc.is_finalized():
        nc.finalize()
    return nc


_NC_CACHE = None


def _get_nc():
    global _NC_CACHE
    if _NC_CACHE is None:
        _NC_CACHE = build_bass()
    return _NC_CACHE


def make_in_maps(x, y, yw, Wq, Wk, Wv, Wp, bp):
    import ml_dtypes

    bf = ml_dtypes.bfloat16
    x = np.asarray(x, np.float32)
    y = np.asarray(y, np.float32)
    yw = np.asarray(yw, np.float32)
    wqT = np.ascontiguousarray(np.asarray(Wq, np.float32).T).astype(bf)
    wkT = np.ascontiguousarray(np.asarray(Wk, np.float32).T).astype(bf)
    wvT = np.ascontiguousarray(np.asarray(Wv, np.float32).T).astype(bf)
    wpT = np.ascontiguousarray(np.asarray(Wp, np.float32).T).astype(bf)
    bpf = np.asarray(bp, np.float32).reshape(1, C)

    in_maps = []
    for c in range(N_CORES):
        b, half = divmod(c, 2)
        n0 = half * NSH
        in_maps.append(
            {
                "xT": np.ascontiguousarray(x[b, n0:n0 + NSH, :].T).astype(bf),
                "yT": np.ascontiguousarray(y[b].T).astype(bf),
                "ywf": np.ascontiguousarray(yw[b].reshape(1, M)),
                "wqT": wqT,
                "wkT": wkT,
                "wvT": wvT,
                "wpT": wpT,
                "bpf": bpf,
            }
        )
    return in_maps


def run(inputs, trace=False):
    """Returns (full_output, BassKernelResults)."""
    from concourse.bass_utils import run_bass_kernel_spmd

    nc = _get_nc()
    in_maps = make_in_maps(**inputs)
    res = run_bass_kernel_spmd(
        nc, in_maps, list(range(N_CORES)), trace=trace
    )
    full = np.empty((B, N, C), dtype=np.float32)
    for c in range(N_CORES):
        b, half = divmod(c, 2)
        n0 = half * NSH
        full[b, n0:n0 + NSH, :] = res.results[c]["out"]
    return full, res


def kernel(**inputs):
    full, _ = run(inputs, trace=False)
    return full


# revision 9
# speedup vs baseline: 1.8425x; 1.0367x over previous
"""Trainium2 Bass kernel for nn_CrossAttention (B=4, N=M=1024, C=768, H=12, D=64).

Sharding: pure data-parallel over 8 cores. Core c handles batch b = c // 2 and
query rows [512*(c%2), 512*(c%2)+512). Each core computes K/V for its batch
(duplicated across the 2 cores sharing a batch) so no collectives are needed.

All-bf16 datapath (fp32 PSUM accumulation); bf16 streams the PE at 1 cycle/row
and avoids the fp32r power throttle. Host-side layout:
  xT  [768, 512]   = x[b, n0:n0+512, :].T   (c-major for Q projection)
  yT  [768, 1024]  = y[b].T                 (c-major for K/V projection)
  w*T [768, 768]   = W.T                    (c-major weights), yw/bp fp32 rows

Device dataflow (all matmuls bf16 x bf16 -> fp32 PSUM):
  QT[co,n] = sum_c wqT[c,co] xT[c,n]
  KT[co,m] = sum_c wkT[c,co] yT[c,m]; + yw via DVE add of a DMA-replicated
             [128,1024] yw row (no bias matmuls)
  V[m,cv]  = sum_c yT[c,m] wvT[c,cv]  in a [128, 12, 128] per-chunk layout
             whose cols 64:128 are memset to 1 so the PV matmul's PSUM rows
             64:128 accumulate Z replicated 64x (softmax denominator, free)
  per head PAIR: two K=64 S-matmuls at PE array tile positions (0,0)/(64,0)
  (they execute concurrently on disjoint sub-arrays) into one [128,1024]
  PSUM tile, ONE exp over [128,1024] -> bf16, two PV matmuls.
  1/Z = stock DVE reciprocal on PSUM rows 64:128 (reciprocal_approx_fast is
  broken on this hardware), DVE multiply -> OT bf16.
  out[n,co] = sum_ci OT[ci,n] wpT[ci,co]; + bp via DVE add (replicated row).

Schedule (the part that matters for the clock): the PE must never idle long
enough for the HAM activity monitor to re-throttle it to 1.2 GHz.
  - ~10 us of throwaway warm-up matmuls at t=0 cover the DMA load phase.
  - Block hp runs head-pair hp's 8-chunk attention loop with head-pair
    hp+1's Q/K projection groups interleaved at chunks 1/3/5 (and all of
    the V projection interleaved into block 0), so the PE always has
    independent work while ACT computes exps.
  - DVE queue order per block: projection casts first, then the previous
    pair's reciprocals/multiplies -- the casts gate the next block's
    S-matmuls; the reciprocals gate only the final output projection.
  - Output projection runs in two 2-bank PSUM waves, ci=5 last, so only
    the last pair's normalize sits on the critical path.
"""

import sys

for _p in ("/opt/trn_rl_repo",):
    if _p not in sys.path:
        sys.path.insert(0, _p)

import numpy as np
from contextlib import ExitStack

import concourse.bass as bass
import concourse.mybir as mybir
import concourse.tile as tile
from concourse import bacc

F32 = mybir.dt.float32
BF16 = mybir.dt.bfloat16

B = 4
N = 1024
M = 1024
C = 768
H = 12
D = 64
NSH = 512            # query rows per core
CK = C // 128        # 6 chunks of the feature dim
MK = M // 128        # 8 chunks of the key dim
HP = H // 2          # 6 head pairs (one KT/QT co-chunk each)
SCALE = D ** -0.5
N_CORES = 8
N_WARM = 24          # warm-up matmuls to keep HAM at 8/8 during loads


def build_bass():
    nc = bacc.Bacc("TRN2", target_bir_lowering=False, debug=False)

    xT = nc.dram_tensor("xT", [C, NSH], BF16, kind="ExternalInput").ap()
    yT = nc.dram_tensor("yT", [C, M], BF16, kind="ExternalInput").ap()
    ywf = nc.dram_tensor("ywf", [1, M], F32, kind="ExternalInput").ap()
    wqT = nc.dram_tensor("wqT", [C, C], BF16, kind="ExternalInput").ap()
    wkT = nc.dram_tensor("wkT", [C, C], BF16, kind="ExternalInput").ap()
    wvT = nc.dram_tensor("wvT", [C, C], BF16, kind="ExternalInput").ap()
    wpT = nc.dram_tensor("wpT", [C, C], BF16, kind="ExternalInput").ap()
    bpf = nc.dram_tensor("bpf", [1, C], F32, kind="ExternalInput").ap()
    out = nc.dram_tensor("out", [NSH, C], F32, kind="ExternalOutput").ap()

    # p-major views so each tensor loads in one (or two) large DMAs
    wq_r = wqT.rearrange("(k p) n -> p k n", p=128)
    wk_r = wkT.rearrange("(k p) n -> p k n", p=128)
    wv_r = wvT.rearrange("(k p) n -> p k n", p=128)
    wp_r = wpT.rearrange("(k p) n -> p k n", p=128)
    xT_r = xT.rearrange("(k p) n -> p k n", p=128)
    yT_r = yT.rearrange("(k p) n -> p k n", p=128)
    out_c = out.rearrange("(k p) n -> k p n", p=128)

    with tile.TileContext(nc) as tc, ExitStack() as ctx:
        wpool = ctx.enter_context(tc.tile_pool(name="w", bufs=4))
        cpool = ctx.enter_context(tc.tile_pool(name="const", bufs=1))
        qpool = ctx.enter_context(tc.tile_pool(name="qt", bufs=3))
        kpool = ctx.enter_context(tc.tile_pool(name="kt", bufs=3))
        vpool = ctx.enter_context(tc.tile_pool(name="vs", bufs=MK))
        opool = ctx.enter_context(tc.tile_pool(name="ot", bufs=CK))
        epool = ctx.enter_context(tc.tile_pool(name="es", bufs=3))
        outpool = ctx.enter_context(tc.tile_pool(name="outs", bufs=2))
        zpool = ctx.enter_context(tc.tile_pool(name="z", bufs=4))
        ppool = ctx.enter_context(tc.tile_pool(name="pp", bufs=2, space="PSUM"))
        oppool = ctx.enter_context(tc.tile_pool(name="op", bufs=4, space="PSUM"))

        # ---- PE warm-up: throwaway matmuls with no DMA dependency ----
        wrm = cpool.tile([128, 512], BF16, tag="wrm")
        nc.gpsimd.memset(wrm, 0.0)
        wps = ppool.tile([128, 1024], F32, tag="pp", name="warmps")
        for i in range(N_WARM):
            nc.tensor.matmul(
                wps[:, 0:512], wrm[:, 0:128], wrm,
                start=(i == 0), stop=(i == N_WARM - 1),
            )

        # ---- input loads; order matters (FIFO per HWDGE ring) ----
        # scalar (qActDynamicHW) ring: wq in 2 halves (co 0:3 first), yT, wp
        # sync   (qSPDynamicHW)  ring: xT, yw row, wk, wv, bp row
        wq = wpool.tile([128, CK, C], BF16, tag="w", name="wq")
        nc.scalar.dma_start(out=wq[:, :, 0:384], in_=wq_r[:, :, 0:384])
        xt = cpool.tile([128, CK, NSH], BF16, tag="xt")
        nc.sync.dma_start(out=xt, in_=xT_r)
        nc.scalar.dma_start(out=wq[:, :, 384:768], in_=wq_r[:, :, 384:768])
        ywb = cpool.tile([128, M], F32, tag="ywb")
        nc.sync.dma_start(
            out=ywb,
            in_=bass.AP(tensor=ywf.tensor, offset=0, ap=[[0, 128], [1, M]]),
        )
        yt = cpool.tile([128, CK, M], BF16, tag="yt")
        nc.scalar.dma_start(out=yt, in_=yT_r)
        wk = wpool.tile([128, CK, C], BF16, tag="w", name="wk")
        nc.sync.dma_start(out=wk, in_=wk_r)
        wv = wpool.tile([128, CK, C], BF16, tag="w", name="wv")
        nc.sync.dma_start(out=wv, in_=wv_r)
        wp = wpool.tile([128, CK, C], BF16, tag="w", name="wp")
        nc.scalar.dma_start(out=wp, in_=wp_r)
        bpb = cpool.tile([128, C], F32, tag="bpb")
        nc.sync.dma_start(
            out=bpb,
            in_=bass.AP(tensor=bpf.tensor, offset=0, ap=[[0, 128], [1, C]]),
        )
        # preload the ACT exp table off the critical path
        warm = cpool.tile([1, 8], F32, tag="warm")
        nc.scalar.activation(
            warm, ywb[0:1, 0:8], mybir.ActivationFunctionType.Exp, scale=SCALE
        )

        # ---- projection pieces (each returns PE-emit + DVE-emit closures
        #      so the matmul groups can be interleaved into attention) ----
        def qproj_mm(co):
            ps = ppool.tile([128, 1024], F32, tag="pp")
            for ci in range(CK):
                nc.tensor.matmul(
                    ps[:, 0:512],
                    wq[:, ci, co * 128:(co + 1) * 128],
                    xt[:, ci, :],
                    start=(ci == 0),
                    stop=(ci == CK - 1),
                )
            t = qpool.tile([128, NSH], BF16, tag="qt")
            nc.vector.tensor_copy(t, ps[:, 0:512])
            return t

        def kproj_mm(co, t, mh):
            ps = ppool.tile([128, 1024], F32, tag="pp")
            sl = slice(mh * 512, (mh + 1) * 512)
            for ci in range(CK):
                nc.tensor.matmul(
                    ps[:, 0:512],
                    wk[:, ci, co * 128:(co + 1) * 128],
                    yt[:, ci, sl],
                    start=(ci == 0),
                    stop=(ci == CK - 1),
                )
            nc.vector.tensor_tensor(
                t[:, sl], ps[:, 0:512], ywb[:, sl], mybir.AluOpType.add
            )

        def vproj(mc):
            t = vpool.tile([128, H, 128], BF16, tag="vs")
            nc.gpsimd.memset(t[:, :, 64:128], 1.0)
            ps = ppool.tile([128, 1024], F32, tag="pp")
            for nh in range(2):
                sl = slice(nh * 512, nh * 512 + 384)
                for ci in range(CK):
                    nc.tensor.matmul(
                        ps[:, sl],
                        yt[:, ci, mc * 128:(mc + 1) * 128],
                        wv[:, ci, nh * 384:(nh + 1) * 384],
                        start=(ci == 0),
                        stop=(ci == CK - 1),
                    )
            for nh in range(2):
                src = ps[:, nh * 512:nh * 512 + 384].rearrange(
                    "p (h e) -> p h e", e=64
                )
                nc.vector.tensor_copy(t[:, nh * 6:(nh + 1) * 6, 0:64], src)
            return t

        vt = [None] * MK
        ot = [None] * HP
        qt = [None] * HP
        kt = [None] * HP

        def attn_block(hp, build_v, build_next):
            """Head-pair hp's attention; next pair's projections (and, for
            hp==0, the V projection) interleaved into the chunk loop."""
            h0, h1 = 2 * hp, 2 * hp + 1
            qtile, ktile = qt[hp], kt[hp]
            op0 = oppool.tile([128, 512], F32, tag="op", name=f"op{h0}")
            op1 = oppool.tile([128, 512], F32, tag="op", name=f"op{h1}")
            nxt = hp + 1
            for mc in range(MK):
                if build_v:
                    vt[mc] = vproj(mc)
                sp = ppool.tile([128, 1024], F32, tag="pp")
                nc.tensor.matmul(
                    sp[:, 0:512],
                    ktile[0:64, mc * 128:(mc + 1) * 128],
                    qtile[0:64, :],
                    start=True,
                    stop=True,
                )
                nc.tensor.matmul(
                    sp[:, 512:1024],
                    ktile[64:128, mc * 128:(mc + 1) * 128],
                    qtile[64:128, :],
                    start=True,
                    stop=True,
                )
                es = epool.tile([128, 1024], BF16, tag="es")
                nc.scalar.activation(
                    es, sp, mybir.ActivationFunctionType.Exp, scale=SCALE
                )
                nc.tensor.matmul(
                    op0, vt[mc][:, h0, :], es[:, 0:512],
                    start=(mc == 0), stop=(mc == MK - 1),
                )
                nc.tensor.matmul(
                    op1, vt[mc][:, h1, :], es[:, 512:1024],
                    start=(mc == 0), stop=(mc == MK - 1),
                )
                if build_next:
                    if mc == 1:
                        qt[nxt] = qproj_mm(nxt)
                    elif mc == 3:
                        kt[nxt] = kpool.tile(
                            [128, M], BF16, tag="kt", name=f"kt{nxt}"
                        )
                        kproj_mm(nxt, kt[nxt], 0)
                    elif mc == 5:
                        kproj_mm(nxt, kt[nxt], 1)
            return op0, op1

        def normalize(hp, op0, op1):
            t = opool.tile([128, NSH], BF16, tag="ot", name=f"ot{hp}")
            for j, op in enumerate((op0, op1)):
                zr = zpool.tile([64, 512], F32, tag="z")
                nc.vector.reciprocal(zr, op[64:128, :])
                nc.vector.tensor_tensor(
                    t[j * 64:(j + 1) * 64, :], op[0:64, :], zr,
                    mybir.AluOpType.mult,
                )
            ot[hp] = t

        qt[0] = qproj_mm(0)
        kt[0] = kpool.tile([128, M], BF16, tag="kt", name="kt0")
        kproj_mm(0, kt[0], 0)
        kproj_mm(0, kt[0], 1)
        prev = None
        for hp in range(HP):
            ops = attn_block(hp, build_v=(hp == 0), build_next=(hp < HP - 1))
            if prev is not None:
                normalize(hp - 1, *prev)
            prev = ops
        normalize(HP - 1, *prev)

        # ---- output projection + bias: two 2-bank waves, ci=5 last ----
        for wave in range(2):
            pss = []
            for n4 in (2 * wave, 2 * wave + 1):
                ps = ppool.tile([128, 1024], F32, tag="pp")
                pss.append(ps)
                for nh in range(2):
                    sl = slice(nh * 512, nh * 512 + 384)
                    for ci in range(CK - 1):
                        nc.tensor.matmul(
                            ps[:, sl],
                            ot[ci][:, n4 * 128:(n4 + 1) * 128],
                            wp[:, ci, nh * 384:(nh + 1) * 384],
                            start=(ci == 0),
                            stop=False,
                        )
            for i, n4 in enumerate((2 * wave, 2 * wave + 1)):
                ps = pss[i]
                for nh in range(2):
                    sl = slice(nh * 512, nh * 512 + 384)
                    nc.tensor.matmul(
                        ps[:, sl],
                        ot[CK - 1][:, n4 * 128:(n4 + 1) * 128],
                        wp[:, CK - 1, nh * 384:(nh + 1) * 384],
                        start=False,
                        stop=True,
                    )
                outs = outpool.tile([128, C], F32, tag="outs")
                for nh in range(2):
                    nc.vector.tensor_tensor(
                        outs[:, nh * 384:(nh + 1) * 384],
                        ps[:, nh * 512:nh * 512 + 384],
                        bpb[:, nh * 384:(nh + 1) * 384],
                        mybir.AluOpType.add,
                    )
                nc.sync.dma_start(out=out_c[n4], in_=outs)

    if not nc.is_finalized():
        nc.finalize()
    return nc


_NC_CACHE = None


def _get_nc():
    global _NC_CACHE
    if _NC_CACHE is None:
        _NC_CACHE = build_bass()
    return _NC_CACHE


def make_in_maps(x, y, yw, Wq, Wk, Wv, Wp, bp):
    import ml_dtypes

    bf = ml_dtypes.bfloat16
    x = np.asarray(x, np.float32)
    y = np.asarray(y, np.float32)
    yw = np.asarray(yw, np.float32)
    wqT = np.ascontiguousarray(np.asarray(Wq, np.float32).T).astype(bf)
    wkT = np.ascontiguousarray(np.asarray(Wk, np.float32).T).astype(bf)
    wvT = np.ascontiguousarray(np.asarray(Wv, np.float32).T).astype(bf)
    wpT = np.ascontiguousarray(np.asarray(Wp, np.float32).T).astype(bf)
    bpf = np.asarray(bp, np.float32).reshape(1, C)

    in_maps = []
    for c in range(N_CORES):
        b, half = divmod(c, 2)
        n0 = half * NSH
        in_maps.append(
            {
                "xT": np.ascontiguousarray(x[b, n0:n0 + NSH, :].T).astype(bf),
                "yT": np.ascontiguousarray(y[b].T).astype(bf),
                "ywf": np.ascontiguousarray(yw[b].reshape(1, M)),
                "wqT": wqT,
                "wkT": wkT,
                "wvT": wvT,
                "wpT": wpT,
                "bpf": bpf,
            }
        )
    return in_maps


def run(inputs, trace=False):
    """Returns (full_output, BassKernelResults)."""
    from concourse.bass_utils import run_bass_kernel_spmd

    nc = _get_nc()
    in_maps = make_in_maps(**inputs)
    res = run_bass_kernel_spmd(
        nc, in_maps, list(range(N_CORES)), trace=trace
    )
    full = np.empty((B, N, C), dtype=np.float32)
    for c in range(N_CORES):
        b, half = divmod(c, 2)
        n0 = half * NSH
        full[b, n0:n0 + NSH, :] = res.results[c]["out"]
    return full, res


def kernel(**inputs):
    full, _ = run(inputs, trace=False)
    return full


# revision 10
# speedup vs baseline: 2.0064x; 1.0890x over previous
"""Trainium2 Bass kernel for nn_CrossAttention (B=4, N=M=1024, C=768, H=12, D=64).

Sharding: pure data-parallel over 8 cores. Core c handles batch b = c // 2 and
query rows [512*(c%2), 512*(c%2)+512). Each core computes K/V for its batch
(duplicated across the 2 cores sharing a batch) so no collectives are needed.

All-bf16 datapath (fp32 PSUM accumulation); bf16 streams the PE at 1 cycle/row
and avoids the fp32r power throttle. Host-side layout:
  xT  [768, 512]   = x[b, n0:n0+512, :].T   (c-major for Q projection)
  yT  [768, 1024]  = y[b].T                 (c-major for K/V projection)
  w*T [768, 768]   = W.T                    (c-major weights)
  ywr [1, 1024]    = yw[b] row (bf16), bp fp32 row (DMA-replicated to 128
                     partitions for the DVE bias add)

Device dataflow (all matmuls bf16 x bf16 -> fp32 PSUM):
  QT[co,n] = sum_c wqT[c,co] xT[c,n]
  KT[co,m] = sum_c wkT[c,co] yT[c,m] + ones-row x ywr (rank-1 bias matmul)
  V[m,cv]  = sum_c yT[c,m] wvT[c,cv]  in a [128, 12, 128] per-chunk layout
             whose cols 64:128 are memset to 1 so the PV matmul's PSUM rows
             64:128 accumulate Z replicated 64x (softmax denominator, free)
  per head PAIR: two K=64 S-matmuls at PE array tile positions (0,0)/(64,0)
  (they execute concurrently on disjoint sub-arrays) into one [128,1024]
  PSUM tile, ONE exp over [128,1024] -> bf16, two PV matmuls.
  1/Z = stock DVE reciprocal on PSUM rows 64:128 (reciprocal_approx_fast is
  broken on this hardware), DVE multiply -> OT bf16.
  out[n,co] = sum_ci OT[ci,n] wpT[ci,co]; + bp via DVE add (replicated row).

Schedule notes (what the trace iterations taught us):
  - DMA: only use APs whose inner contiguous run is large. Each [128, 768]
    weight chunk of a (k p) n view is one linear 393KB region; the p-major
    single-DMA variant (768B segments) runs at ~2 GB/s/engine and starves
    everything.
  - ~8.5 us of throwaway warm-up matmuls at t=0 keep the HAM activity
    monitor at K=8/8 (2.4 GHz) while the weights load.
  - Block hp runs head-pair hp's 8-chunk attention with head-pair hp+1's
    Q/K projections interleaved at chunks 1/3/5 (V projection inside block
    0), so the PE never idles long enough to re-throttle.
  - Projection PSUM->SBUF casts run on the SCALAR engine: they land in the
    exp FIFO right where the PSUM pool rotation needs them, and keep the
    DVE free for the reciprocals (GpSimd cannot read PSUM; a DVE cast
    behind a 3.4 us reciprocal stalled the PE a full 7 us per block).
  - Output projection runs in two 2-bank PSUM waves, ci=5 last, so only
    the last pair's normalize sits on the critical path.
"""

import sys

for _p in ("/opt/trn_rl_repo",):
    if _p not in sys.path:
        sys.path.insert(0, _p)

import numpy as np
from contextlib import ExitStack

import concourse.bass as bass
import concourse.mybir as mybir
import concourse.tile as tile
from concourse import bacc

F32 = mybir.dt.float32
BF16 = mybir.dt.bfloat16

B = 4
N = 1024
M = 1024
C = 768
H = 12
D = 64
NSH = 512            # query rows per core
CK = C // 128        # 6 chunks of the feature dim
MK = M // 128        # 8 chunks of the key dim
HP = H // 2          # 6 head pairs (one KT/QT co-chunk each)
SCALE = D ** -0.5
N_CORES = 8
N_WARM = 20          # warm-up matmuls to keep HAM at 8/8 during loads


def build_bass():
    nc = bacc.Bacc("TRN2", target_bir_lowering=False, debug=False)

    xT = nc.dram_tensor("xT", [C, NSH], BF16, kind="ExternalInput").ap()
    yT = nc.dram_tensor("yT", [C, M], BF16, kind="ExternalInput").ap()
    ywr = nc.dram_tensor("ywr", [1, M], BF16, kind="ExternalInput").ap()
    wqT = nc.dram_tensor("wqT", [C, C], BF16, kind="ExternalInput").ap()
    wkT = nc.dram_tensor("wkT", [C, C], BF16, kind="ExternalInput").ap()
    wvT = nc.dram_tensor("wvT", [C, C], BF16, kind="ExternalInput").ap()
    wpT = nc.dram_tensor("wpT", [C, C], BF16, kind="ExternalInput").ap()
    bpf = nc.dram_tensor("bpf", [1, C], F32, kind="ExternalInput").ap()
    out = nc.dram_tensor("out", [NSH, C], F32, kind="ExternalOutput").ap()

    # k-major chunk views: each [128, x] chunk is one contiguous DRAM region
    wq_c = wqT.rearrange("(k p) n -> k p n", p=128)
    wk_c = wkT.rearrange("(k p) n -> k p n", p=128)
    wv_c = wvT.rearrange("(k p) n -> k p n", p=128)
    wp_c = wpT.rearrange("(k p) n -> k p n", p=128)
    xT_c = xT.rearrange("(k p) n -> k p n", p=128)
    yT_c = yT.rearrange("(k p) n -> k p n", p=128)
    out_c = out.rearrange("(k p) n -> k p n", p=128)

    with tile.TileContext(nc) as tc, ExitStack() as ctx:
        wpool = ctx.enter_context(tc.tile_pool(name="w", bufs=4))
        cpool = ctx.enter_context(tc.tile_pool(name="const", bufs=1))
        qpool = ctx.enter_context(tc.tile_pool(name="qt", bufs=3))
        kpool = ctx.enter_context(tc.tile_pool(name="kt", bufs=3))
        vpool = ctx.enter_context(tc.tile_pool(name="vs", bufs=MK))
        opool = ctx.enter_context(tc.tile_pool(name="ot", bufs=CK))
        epool = ctx.enter_context(tc.tile_pool(name="es", bufs=3))
        outpool = ctx.enter_context(tc.tile_pool(name="outs", bufs=2))
        zpool = ctx.enter_context(tc.tile_pool(name="z", bufs=4))
        ppool = ctx.enter_context(tc.tile_pool(name="pp", bufs=2, space="PSUM"))
        oppool = ctx.enter_context(tc.tile_pool(name="op", bufs=4, space="PSUM"))

        # ---- PE warm-up: throwaway matmuls with no DMA dependency ----
        wrm = cpool.tile([128, 512], BF16, tag="wrm")
        nc.gpsimd.memset(wrm, 0.0)
        ones = cpool.tile([1, C], BF16, tag="ones")
        nc.gpsimd.memset(ones, 1.0)
        wps = ppool.tile([128, 1024], F32, tag="pp", name="warmps")
        for i in range(N_WARM):
            nc.tensor.matmul(
                wps[:, 0:512], wrm[:, 0:128], wrm,
                start=(i == 0), stop=(i == N_WARM - 1),
            )

        # ---- input loads: contiguous per-chunk DMAs, 2 HWDGE rings ----
        # scalar ring: wq x6 (Q proj gate), yT x6, wp x6
        # sync ring:   xt x6, yw row, wk x6, wv x6, bp replicate
        wq = wpool.tile([128, CK, C], BF16, tag="w", name="wq")
        xt = cpool.tile([128, CK, NSH], BF16, tag="xt")
        for i in range(CK):
            nc.scalar.dma_start(out=wq[:, i, :], in_=wq_c[i])
            nc.sync.dma_start(out=xt[:, i, :], in_=xT_c[i])
        yw_s = cpool.tile([1, M], BF16, tag="yws")
        nc.sync.dma_start(out=yw_s, in_=ywr)
        yt = cpool.tile([128, CK, M], BF16, tag="yt")
        wk = wpool.tile([128, CK, C], BF16, tag="w", name="wk")
        for i in range(CK):
            nc.scalar.dma_start(out=yt[:, i, :], in_=yT_c[i])
            nc.sync.dma_start(out=wk[:, i, :], in_=wk_c[i])
        wv = wpool.tile([128, CK, C], BF16, tag="w", name="wv")
        for i in range(CK):
            nc.sync.dma_start(out=wv[:, i, :], in_=wv_c[i])
        wp = wpool.tile([128, CK, C], BF16, tag="w", name="wp")
        for i in range(CK):
            nc.scalar.dma_start(out=wp[:, i, :], in_=wp_c[i])
        bpb = cpool.tile([128, C], F32, tag="bpb")
        nc.sync.dma_start(
            out=bpb,
            in_=bass.AP(tensor=bpf.tensor, offset=0, ap=[[0, 128], [1, C]]),
        )
        # preload the ACT exp table off the critical path
        warm = cpool.tile([1, 8], F32, tag="warm")
        nc.scalar.activation(
            warm, bpb[0:1, 0:8], mybir.ActivationFunctionType.Exp, scale=SCALE
        )

        def qproj_mm(co):
            ps = ppool.tile([128, 1024], F32, tag="pp")
            for ci in range(CK):
                nc.tensor.matmul(
                    ps[:, 0:512],
                    wq[:, ci, co * 128:(co + 1) * 128],
                    xt[:, ci, :],
                    start=(ci == 0),
                    stop=(ci == CK - 1),
                )
            t = qpool.tile([128, NSH], BF16, tag="qt")
            nc.scalar.copy(t, ps[:, 0:512])
            return t

        def kproj_mm(co, t, mh):
            ps = ppool.tile([128, 1024], F32, tag="pp")
            sl = slice(mh * 512, (mh + 1) * 512)
            for ci in range(CK):
                nc.tensor.matmul(
                    ps[:, 0:512],
                    wk[:, ci, co * 128:(co + 1) * 128],
                    yt[:, ci, sl],
                    start=(ci == 0),
                    stop=False,
                )
            # += ones-row^T x yw-row: the additive key bias, rank-1
            nc.tensor.matmul(
                ps[:, 0:512],
                ones[:, co * 128:(co + 1) * 128],
                yw_s[:, sl],
                start=False,
                stop=True,
            )
            nc.scalar.copy(t[:, sl], ps[:, 0:512])

        def vproj(mc):
            t = vpool.tile([128, H, 128], BF16, tag="vs")
            nc.gpsimd.memset(t[:, :, 64:128], 1.0)
            ps = ppool.tile([128, 1024], F32, tag="pp")
            for nh in range(2):
                sl = slice(nh * 512, nh * 512 + 384)
                for ci in range(CK):
                    nc.tensor.matmul(
                        ps[:, sl],
                        yt[:, ci, mc * 128:(mc + 1) * 128],
                        wv[:, ci, nh * 384:(nh + 1) * 384],
                        start=(ci == 0),
                        stop=(ci == CK - 1),
                    )
            for nh in range(2):
                src = ps[:, nh * 512:nh * 512 + 384].rearrange(
                    "p (h e) -> p h e", e=64
                )
                nc.vector.tensor_copy(t[:, nh * 6:(nh + 1) * 6, 0:64], src)
            return t

        vt = [None] * MK
        ot = [None] * HP
        qt = [None] * HP
        kt = [None] * HP

        def attn_block(hp, build_v, build_next):
            """Head-pair hp's attention; next pair's projections (and, for
            hp==0, the V projection) interleaved into the chunk loop."""
            h0, h1 = 2 * hp, 2 * hp + 1
            qtile, ktile = qt[hp], kt[hp]
            op0 = oppool.tile([128, 512], F32, tag="op", name=f"op{h0}")
            op1 = oppool.tile([128, 512], F32, tag="op", name=f"op{h1}")
            nxt = hp + 1
            for mc in range(MK):
                if build_v:
                    vt[mc] = vproj(mc)
                sp = ppool.tile([128, 1024], F32, tag="pp")
                nc.tensor.matmul(
                    sp[:, 0:512],
                    ktile[0:64, mc * 128:(mc + 1) * 128],
                    qtile[0:64, :],
                    start=True,
                    stop=True,
                )
                nc.tensor.matmul(
                    sp[:, 512:1024],
                    ktile[64:128, mc * 128:(mc + 1) * 128],
                    qtile[64:128, :],
                    start=True,
                    stop=True,
                )
                es = epool.tile([128, 1024], BF16, tag="es")
                nc.scalar.activation(
                    es, sp, mybir.ActivationFunctionType.Exp, scale=SCALE
                )
                nc.tensor.matmul(
                    op0, vt[mc][:, h0, :], es[:, 0:512],
                    start=(mc == 0), stop=(mc == MK - 1),
                )
                nc.tensor.matmul(
                    op1, vt[mc][:, h1, :], es[:, 512:1024],
                    start=(mc == 0), stop=(mc == MK - 1),
                )
                if build_next:
                    if mc == 1:
                        qt[nxt] = qproj_mm(nxt)
                    elif mc == 3:
                        kt[nxt] = kpool.tile(
                            [128, M], BF16, tag="kt", name=f"kt{nxt}"
                        )
                        kproj_mm(nxt, kt[nxt], 0)
                    elif mc == 5:
                        kproj_mm(nxt, kt[nxt], 1)
            return op0, op1

        def normalize(hp, op0, op1):
            t = opool.tile([128, NSH], BF16, tag="ot", name=f"ot{hp}")
            for j, op in enumerate((op0, op1)):
                zr = zpool.tile([64, 512], F32, tag="z")
                nc.vector.reciprocal(zr, op[64:128, :])
                nc.vector.tensor_tensor(
                    t[j * 64:(j + 1) * 64, :], op[0:64, :], zr,
                    mybir.AluOpType.mult,
                )
            ot[hp] = t

        qt[0] = qproj_mm(0)
        kt[0] = kpool.tile([128, M], BF16, tag="kt", name="kt0")
        kproj_mm(0, kt[0], 0)
        kproj_mm(0, kt[0], 1)
        prev = None
        for hp in range(HP):
            ops = attn_block(hp, build_v=(hp == 0), build_next=(hp < HP - 1))
            if prev is not None:
                normalize(hp - 1, *prev)
            prev = ops
        normalize(HP - 1, *prev)

        # ---- output projection + bias: two 2-bank waves, ci=5 last ----
        for wave in range(2):
            pss = []
            for n4 in (2 * wave, 2 * wave + 1):
                ps = ppool.tile([128, 1024], F32, tag="pp")
                pss.append(ps)
                for nh in range(2):
                    sl = slice(nh * 512, nh * 512 + 384)
                    for ci in range(CK - 1):
                        nc.tensor.matmul(
                            ps[:, sl],
                            ot[ci][:, n4 * 128:(n4 + 1) * 128],
                            wp[:, ci, nh * 384:(nh + 1) * 384],
                            start=(ci == 0),
                            stop=False,
                        )
            for i, n4 in enumerate((2 * wave, 2 * wave + 1)):
                ps = pss[i]
                for nh in range(2):
                    sl = slice(nh * 512, nh * 512 + 384)
                    nc.tensor.matmul(
                        ps[:, sl],
                        ot[CK - 1][:, n4 * 128:(n4 + 1) * 128],
                        wp[:, CK - 1, nh * 384:(nh + 1) * 384],
                        start=False,
                        stop=True,
                    )
                outs = outpool.tile([128, C], F32, tag="outs")
                for nh in range(2):
                    nc.vector.tensor_tensor(
                        outs[:, nh * 384:(nh + 1) * 384],
                        ps[:, nh * 512:nh * 512 + 384],
                        bpb[:, nh * 384:(nh + 1) * 384],
                        mybir.AluOpType.add,
                    )
                nc.sync.dma_start(out=out_c[n4], in_=outs)

    if not nc.is_finalized():
        nc.finalize()
    return nc


_NC_CACHE = None


def _get_nc():
    global _NC_CACHE
    if _NC_CACHE is None:
        _NC_CACHE = build_bass()
    return _NC_CACHE


def make_in_maps(x, y, yw, Wq, Wk, Wv, Wp, bp):
    import ml_dtypes

    bf = ml_dtypes.bfloat16
    x = np.asarray(x, np.float32)
    y = np.asarray(y, np.float32)
    yw = np.asarray(yw, np.float32)
    wqT = np.ascontiguousarray(np.asarray(Wq, np.float32).T).astype(bf)
    wkT = np.ascontiguousarray(np.asarray(Wk, np.float32).T).astype(bf)
    wvT = np.ascontiguousarray(np.asarray(Wv, np.float32).T).astype(bf)
    wpT = np.ascontiguousarray(np.asarray(Wp, np.float32).T).astype(bf)
    bpf = np.asarray(bp, np.float32).reshape(1, C)

    in_maps = []
    for c in range(N_CORES):
        b, half = divmod(c, 2)
        n0 = half * NSH
        in_maps.append(
            {
                "xT": np.ascontiguousarray(x[b, n0:n0 + NSH, :].T).astype(bf),
                "yT": np.ascontiguousarray(y[b].T).astype(bf),
                "ywr": np.ascontiguousarray(yw[b].reshape(1, M)).astype(bf),
                "wqT": wqT,
                "wkT": wkT,
                "wvT": wvT,
                "wpT": wpT,
                "bpf": bpf,
            }
        )
    return in_maps


def run(inputs, trace=False):
    """Returns (full_output, BassKernelResults)."""
    from concourse.bass_utils import run_bass_kernel_spmd

    nc = _get_nc()
    in_maps = make_in_maps(**inputs)
    res = run_bass_kernel_spmd(
        nc, in_maps, list(range(N_CORES)), trace=trace
    )
    full = np.empty((B, N, C), dtype=np.float32)
    for c in range(N_CORES):
        b, half = divmod(c, 2)
        n0 = half * NSH
        full[b, n0:n0 + NSH, :] = res.results[c]["out"]
    return full, res


def kernel(**inputs):
    full, _ = run(inputs, trace=False)
    return full


# revision 15
# speedup vs baseline: 2.0240x; 1.0088x over previous
"""Trainium2 Bass kernel for nn_CrossAttention (B=4, N=M=1024, C=768, H=12, D=64).

Sharding: pure data-parallel over 8 cores. Core c handles batch b = c // 2 and
query rows [512*(c%2), 512*(c%2)+512). Each core computes K/V for its batch
(duplicated across the 2 cores sharing a batch) so no collectives are needed.

All-bf16 datapath (fp32 PSUM accumulation); bf16 streams the PE at 1 cycle/row
and avoids the fp32r power throttle. Host-side layout:
  xT  [768, 512]   = x[b, n0:n0+512, :].T   (c-major for Q projection)
  yT  [768, 1024]  = y[b].T                 (c-major for K/V projection)
  w*T [768, 768]   = W.T                    (c-major weights)
  ywr [1, 1024]    = yw[b] row (bf16), bp fp32 row (DMA-replicated to 128
                     partitions for the DVE bias add)

Device dataflow (all matmuls bf16 x bf16 -> fp32 PSUM):
  QT[co,n] = sum_c wqT[c,co] xT[c,n]
  KT[co,m] = sum_c wkT[c,co] yT[c,m] + ones-row x ywr (rank-1 bias matmul)
  V[m,cv]  = sum_c yT[c,m] wvT[c,cv]  in a [128, 12, 128] per-chunk layout
             whose cols 64:128 are memset to 1 so the PV matmul's PSUM rows
             64:128 accumulate Z replicated 64x (softmax denominator, free)
  per head PAIR: two K=64 S-matmuls at PE array tile positions (0,0)/(64,0)
  (they execute concurrently on disjoint sub-arrays) into one [128,1024]
  PSUM tile, ONE exp over [128,1024] -> bf16, two PV matmuls.
  1/Z = stock DVE reciprocal on PSUM rows 64:128 (reciprocal_approx_fast is
  broken on this hardware), DVE multiply -> OT bf16.
  out[n,co] = sum_ci OT[ci,n] wpT[ci,co]; + bp via DVE add (replicated row).

Schedule notes (what the trace iterations taught us):
  - DMA: only use APs whose inner contiguous run is large. Each [128, 768]
    weight chunk of a (k p) n view is one linear 393KB region; the p-major
    single-DMA variant (768B segments) runs at ~2 GB/s/engine and starves
    everything.
  - ~8.5 us of throwaway warm-up matmuls at t=0 keep the HAM activity
    monitor at K=8/8 (2.4 GHz) while the weights load.
  - Block hp runs head-pair hp's 8-chunk attention with head-pair hp+1's
    Q/K projections interleaved at chunks 1/3/5 (V projection inside block
    0), so the PE never idles long enough to re-throttle.
  - Projection PSUM->SBUF casts run on the SCALAR engine: they land in the
    exp FIFO right where the PSUM pool rotation needs them, and keep the
    DVE free for the reciprocals (GpSimd cannot read PSUM; a DVE cast
    behind a 3.4 us reciprocal stalled the PE a full 7 us per block).
  - Output projection runs in two 2-bank PSUM waves, ci=5 last, so only
    the last pair's normalize sits on the critical path.
"""

import sys

for _p in ("/opt/trn_rl_repo",):
    if _p not in sys.path:
        sys.path.insert(0, _p)

import numpy as np
from contextlib import ExitStack

import concourse.bass as bass
import concourse.mybir as mybir
import concourse.tile as tile
from concourse import bacc

F32 = mybir.dt.float32
BF16 = mybir.dt.bfloat16

B = 4
N = 1024
M = 1024
C = 768
H = 12
D = 64
NSH = 512            # query rows per core
CK = C // 128        # 6 chunks of the feature dim
MK = M // 128        # 8 chunks of the key dim
HP = H // 2          # 6 head pairs (one KT/QT co-chunk each)
SCALE = D ** -0.5
N_CORES = 8
N_WARM = 20          # warm-up matmuls to keep HAM at 8/8 during loads


def build_bass():
    nc = bacc.Bacc("TRN2", target_bir_lowering=False, debug=False)

    xT = nc.dram_tensor("xT", [C, NSH], BF16, kind="ExternalInput").ap()
    yT = nc.dram_tensor("yT", [C, M], BF16, kind="ExternalInput").ap()
    ywr = nc.dram_tensor("ywr", [1, M], BF16, kind="ExternalInput").ap()
    wqT = nc.dram_tensor("wqT", [C, C], BF16, kind="ExternalInput").ap()
    wkT = nc.dram_tensor("wkT", [C, C], BF16, kind="ExternalInput").ap()
    wvT = nc.dram_tensor("wvT", [C, C], BF16, kind="ExternalInput").ap()
    wpT = nc.dram_tensor("wpT", [C, C], BF16, kind="ExternalInput").ap()
    bpf = nc.dram_tensor("bpf", [1, C], F32, kind="ExternalInput").ap()
    out = nc.dram_tensor("out", [NSH, C], F32, kind="ExternalOutput").ap()

    # k-major chunk views: each [128, x] chunk is one contiguous DRAM region
    wq_c = wqT.rearrange("(k p) n -> k p n", p=128)
    wk_c = wkT.rearrange("(k p) n -> k p n", p=128)
    wv_c = wvT.rearrange("(k p) n -> k p n", p=128)
    wp_c = wpT.rearrange("(k p) n -> k p n", p=128)
    xT_c = xT.rearrange("(k p) n -> k p n", p=128)
    yT_c = yT.rearrange("(k p) n -> k p n", p=128)
    out_c = out.rearrange("(k p) n -> k p n", p=128)

    with tile.TileContext(nc) as tc, ExitStack() as ctx:
        wpool = ctx.enter_context(tc.tile_pool(name="w", bufs=4))
        cpool = ctx.enter_context(tc.tile_pool(name="const", bufs=1))
        qpool = ctx.enter_context(tc.tile_pool(name="qt", bufs=3))
        kpool = ctx.enter_context(tc.tile_pool(name="kt", bufs=3))
        vpool = ctx.enter_context(tc.tile_pool(name="vs", bufs=MK))
        opool = ctx.enter_context(tc.tile_pool(name="ot", bufs=CK))
        epool = ctx.enter_context(tc.tile_pool(name="es", bufs=3))
        outpool = ctx.enter_context(tc.tile_pool(name="outs", bufs=2))
        zpool = ctx.enter_context(tc.tile_pool(name="z", bufs=4))
        ppool = ctx.enter_context(tc.tile_pool(name="pp", bufs=2, space="PSUM"))
        oppool = ctx.enter_context(tc.tile_pool(name="op", bufs=4, space="PSUM"))

        # ---- PE warm-up: throwaway matmuls with no DMA dependency ----
        wrm = cpool.tile([128, 512], BF16, tag="wrm")
        nc.gpsimd.memset(wrm, 0.0)
        ones = cpool.tile([1, C], BF16, tag="ones")
        nc.gpsimd.memset(ones, 1.0)
        wps = ppool.tile([128, 1024], F32, tag="pp", name="warmps")
        for i in range(N_WARM):
            nc.tensor.matmul(
                wps[:, 0:512], wrm[:, 0:128], wrm,
                start=(i == 0), stop=(i == N_WARM - 1),
            )

        # ---- input loads: contiguous per-chunk DMAs, 2 HWDGE rings ----
        # scalar ring: wq x6 (Q proj gate), yT x6, wp x6
        # sync ring:   xt x6, yw row, wk x6, wv x6, bp replicate
        wq = wpool.tile([128, CK, C], BF16, tag="w", name="wq")
        xt = cpool.tile([128, CK, NSH], BF16, tag="xt")
        for i in range(CK):
            nc.scalar.dma_start(out=wq[:, i, :], in_=wq_c[i])
            nc.sync.dma_start(out=xt[:, i, :], in_=xT_c[i])
        yw_s = cpool.tile([1, M], BF16, tag="yws")
        nc.sync.dma_start(out=yw_s, in_=ywr)
        yt = cpool.tile([128, CK, M], BF16, tag="yt")
        wk = wpool.tile([128, CK, C], BF16, tag="w", name="wk")
        for i in range(CK):
            nc.scalar.dma_start(out=yt[:, i, :], in_=yT_c[i])
            nc.sync.dma_start(out=wk[:, i, :], in_=wk_c[i])
        wv = wpool.tile([128, CK, C], BF16, tag="w", name="wv")
        for i in range(CK):
            nc.sync.dma_start(out=wv[:, i, :], in_=wv_c[i])
        wp = wpool.tile([128, CK, C], BF16, tag="w", name="wp")
        for i in range(CK):
            nc.scalar.dma_start(out=wp[:, i, :], in_=wp_c[i])
        bpb = cpool.tile([128, C], F32, tag="bpb")
        nc.sync.dma_start(
            out=bpb,
            in_=bass.AP(tensor=bpf.tensor, offset=0, ap=[[0, 128], [1, C]]),
        )
        # preload the ACT exp table off the critical path
        warm = cpool.tile([1, 8], F32, tag="warm")
        nc.scalar.activation(
            warm, bpb[0:1, 0:8], mybir.ActivationFunctionType.Exp, scale=SCALE
        )

        def qproj_mm(co):
            ps = ppool.tile([128, 1024], F32, tag="pp")
            for ci in range(CK):
                nc.tensor.matmul(
                    ps[:, 0:512],
                    wq[:, ci, co * 128:(co + 1) * 128],
                    xt[:, ci, :],
                    start=(ci == 0),
                    stop=(ci == CK - 1),
                )
            t = qpool.tile([128, NSH], BF16, tag="qt")
            nc.scalar.copy(t, ps[:, 0:512])
            return t

        def kproj_mm(co, t, mh):
            ps = ppool.tile([128, 1024], F32, tag="pp")
            sl = slice(mh * 512, (mh + 1) * 512)
            for ci in range(CK):
                nc.tensor.matmul(
                    ps[:, 0:512],
                    wk[:, ci, co * 128:(co + 1) * 128],
                    yt[:, ci, sl],
                    start=(ci == 0),
                    stop=False,
                )
            # += ones-row^T x yw-row: the additive key bias, rank-1
            nc.tensor.matmul(
                ps[:, 0:512],
                ones[:, co * 128:(co + 1) * 128],
                yw_s[:, sl],
                start=False,
                stop=True,
            )
            nc.scalar.copy(t[:, sl], ps[:, 0:512])

        def vproj(mc):
            t = vpool.tile([128, H, 128], BF16, tag="vs")
            nc.gpsimd.memset(t[:, :, 64:128], 1.0)
            ps = ppool.tile([128, 1024], F32, tag="pp")
            for nh in range(2):
                sl = slice(nh * 512, nh * 512 + 384)
                for ci in range(CK):
                    nc.tensor.matmul(
                        ps[:, sl],
                        yt[:, ci, mc * 128:(mc + 1) * 128],
                        wv[:, ci, nh * 384:(nh + 1) * 384],
                        start=(ci == 0),
                        stop=(ci == CK - 1),
                    )
            for nh in range(2):
                src = ps[:, nh * 512:nh * 512 + 384].rearrange(
                    "p (h e) -> p h e", e=64
                )
                nc.vector.tensor_copy(t[:, nh * 6:(nh + 1) * 6, 0:64], src)
            return t

        vt = [None] * MK
        ot = [None] * HP
        qt = [None] * HP
        kt = [None] * HP

        def attn_block(hp, build_v, build_next):
            """Head-pair hp's attention; next pair's projections (and, for
            hp==0, the V projection) sliced into the chunk loop.  The PV
            pair for chunk mc is emitted during chunk mc+1 so the PE never
            sits behind the in-flight exp."""
            h0, h1 = 2 * hp, 2 * hp + 1
            qtile, ktile = qt[hp], kt[hp]
            op0 = oppool.tile([128, 512], F32, tag="op", name=f"op{h0}")
            op1 = oppool.tile([128, 512], F32, tag="op", name=f"op{h1}")
            nxt = hp + 1
            ess = [None] * MK

            def pv(mc):
                nc.tensor.matmul(
                    op0, vt[mc][:, h0, :], ess[mc][:, 0:512],
                    start=(mc == 0), stop=(mc == MK - 1),
                )
                nc.tensor.matmul(
                    op1, vt[mc][:, h1, :], ess[mc][:, 512:1024],
                    start=(mc == 0), stop=(mc == MK - 1),
                )

            def proj_slice(slot):
                if not build_next:
                    return
                if slot == 1:
                    qt[nxt] = qproj_mm(nxt)
                elif slot == 3:
                    kt[nxt] = kpool.tile(
                        [128, M], BF16, tag="kt", name=f"kt{nxt}"
                    )
                    kproj_mm(nxt, kt[nxt], 0)
                elif slot == 5:
                    kproj_mm(nxt, kt[nxt], 1)

            for mc in range(MK):
                if build_v:
                    vt[mc] = vproj(mc)
                sp = ppool.tile([128, 1024], F32, tag="pp")
                nc.tensor.matmul(
                    sp[:, 0:512],
                    ktile[0:64, mc * 128:(mc + 1) * 128],
                    qtile[0:64, :],
                    start=True,
                    stop=True,
                )
                nc.tensor.matmul(
                    sp[:, 512:1024],
                    ktile[64:128, mc * 128:(mc + 1) * 128],
                    qtile[64:128, :],
                    start=True,
                    stop=True,
                )
                es = epool.tile([128, 1024], BF16, tag="es")
                nc.scalar.activation(
                    es, sp, mybir.ActivationFunctionType.Exp, scale=SCALE
                )
                ess[mc] = es
                if mc >= 1:
                    pv(mc - 1)
                proj_slice(mc)
            pv(MK - 1)
            return op0, op1

        def normalize(hp, op0, op1):
            t = opool.tile([128, NSH], BF16, tag="ot", name=f"ot{hp}")
            for j, op in enumerate((op0, op1)):
                zr = zpool.tile([64, 512], F32, tag="z")
                nc.vector.reciprocal(zr, op[64:128, :])
                nc.vector.tensor_tensor(
                    t[j * 64:(j + 1) * 64, :], op[0:64, :], zr,
                    mybir.AluOpType.mult,
                )
            ot[hp] = t

        qt[0] = qproj_mm(0)
        kt[0] = kpool.tile([128, M], BF16, tag="kt", name="kt0")
        kproj_mm(0, kt[0], 0)
        kproj_mm(0, kt[0], 1)
        prev = None
        for hp in range(HP):
            if prev is not None:
                normalize(hp - 1, *prev)
            prev = attn_block(
                hp, build_v=(hp == 0), build_next=(hp < HP - 1)
            )
        normalize(HP - 1, *prev)

        # ---- output projection + bias: two 2-bank waves.  ci=5 runs last
        # as two packed K=64 matmuls so wave A needs only ot[5][0:64]
        # (ready after the first reciprocal); throwaway warm matmuls keep
        # the HAM clock at 8/8 while the last normalize drains. ----
        for wave in range(2):
            pss = []
            for n4 in (2 * wave, 2 * wave + 1):
                ps = ppool.tile([128, 1024], F32, tag="pp")
                pss.append(ps)
                for nh in range(2):
                    sl = slice(nh * 512, nh * 512 + 384)
                    for ci in range(CK - 1):
                        nc.tensor.matmul(
                            ps[:, sl],
                            ot[ci][:, n4 * 128:(n4 + 1) * 128],
                            wp[:, ci, nh * 384:(nh + 1) * 384],
                            start=(ci == 0),
                            stop=False,
                        )
            if wave == 0:
                wop = oppool.tile([128, 512], F32, tag="op", name="tailwarm")
                for i in range(16):
                    nc.tensor.matmul(
                        wop, wrm[:, 0:128], wrm,
                        start=(i == 0), stop=(i == 15),
                    )
            for i, n4 in enumerate((2 * wave, 2 * wave + 1)):
                ps = pss[i]
                for nh in range(2):
                    sl = slice(nh * 512, nh * 512 + 384)
                    nc.tensor.matmul(
                        ps[:, sl],
                        ot[CK - 1][:, n4 * 128:(n4 + 1) * 128],
                        wp[:, CK - 1, nh * 384:(nh + 1) * 384],
                        start=False,
                        stop=True,
                    )
                outs = outpool.tile([128, C], F32, tag="outs")
                for nh in range(2):
                    nc.vector.tensor_tensor(
                        outs[:, nh * 384:(nh + 1) * 384],
                        ps[:, nh * 512:nh * 512 + 384],
                        bpb[:, nh * 384:(nh + 1) * 384],
                        mybir.AluOpType.add,
                    )
                nc.sync.dma_start(out=out_c[n4], in_=outs)

    if not nc.is_finalized():
        nc.finalize()
    return nc


_NC_CACHE = None


def _get_nc():
    global _NC_CACHE
    if _NC_CACHE is None:
        _NC_CACHE = build_bass()
    return _NC_CACHE


def make_in_maps(x, y, yw, Wq, Wk, Wv, Wp, bp):
    import ml_dtypes

    bf = ml_dtypes.bfloat16
    x = np.asarray(x, np.float32)
    y = np.asarray(y, np.float32)
    yw = np.asarray(yw, np.float32)
    wqT = np.ascontiguousarray(np.asarray(Wq, np.float32).T).astype(bf)
    wkT = np.ascontiguousarray(np.asarray(Wk, np.float32).T).astype(bf)
    wvT = np.ascontiguousarray(np.asarray(Wv, np.float32).T).astype(bf)
    wpT = np.ascontiguousarray(np.asarray(Wp, np.float32).T).astype(bf)
    bpf = np.asarray(bp, np.float32).reshape(1, C)

    in_maps = []
    for c in range(N_CORES):
        b, half = divmod(c, 2)
        n0 = half * NSH
        in_maps.append(
            {
                "xT": np.ascontiguousarray(x[b, n0:n0 + NSH, :].T).astype(bf),
                "yT": np.ascontiguousarray(y[b].T).astype(bf),
                "ywr": np.ascontiguousarray(yw[b].reshape(1, M)).astype(bf),
                "wqT": wqT,
                "wkT": wkT,
                "wvT": wvT,
                "wpT": wpT,
                "bpf": bpf,
            }
        )
    return in_maps


def run(inputs, trace=False):
    """Returns (full_output, BassKernelResults)."""
    from concourse.bass_utils import run_bass_kernel_spmd

    nc = _get_nc()
    in_maps = make_in_maps(**inputs)
    res = run_bass_kernel_spmd(
        nc, in_maps, list(range(N_CORES)), trace=trace
    )
    full = np.empty((B, N, C), dtype=np.float32)
    for c in range(N_CORES):
        b, half = divmod(c, 2)
        n0 = half * NSH
        full[b, n0:n0 + NSH, :] = res.results[c]["out"]
    return full, res


def kernel(**inputs):
    full, _ = run(inputs, trace=False)
    return full


# revision 16
# speedup vs baseline: 2.0707x; 1.0231x over previous
"""Trainium2 Bass kernel for nn_CrossAttention (B=4, N=M=1024, C=768, H=12, D=64).

Sharding: pure data-parallel over 8 cores. Core c handles batch b = c // 2 and
query rows [512*(c%2), 512*(c%2)+512). Each core computes K/V for its batch
(duplicated across the 2 cores sharing a batch) so no collectives are needed.

All-bf16 datapath (fp32 PSUM accumulation); bf16 streams the PE at 1 cycle/row
and avoids the fp32r power throttle. Host-side layout:
  xT  [768, 512]   = x[b, n0:n0+512, :].T   (c-major for Q projection)
  yT  [768, 1024]  = y[b].T                 (c-major for K/V projection)
  w*T [768, 768]   = W.T                    (c-major weights)
  ywr [1, 1024]    = yw[b] row (bf16), bp fp32 row (DMA-replicated to 128
                     partitions for the DVE bias add)

Device dataflow (all matmuls bf16 x bf16 -> fp32 PSUM):
  QT[co,n] = sum_c wqT[c,co] xT[c,n]
  KT[co,m] = sum_c wkT[c,co] yT[c,m] + ones-row x ywr (rank-1 bias matmul)
  V[m,cv]  = sum_c yT[c,m] wvT[c,cv]  in a [128, 12, 128] per-chunk layout
             whose cols 64:128 are memset to 1 so the PV matmul's PSUM rows
             64:128 accumulate Z replicated 64x (softmax denominator, free)
  per head PAIR: two K=64 S-matmuls at PE array tile positions (0,0)/(64,0)
  (they execute concurrently on disjoint sub-arrays) into one [128,1024]
  PSUM tile, ONE exp over [128,1024] -> bf16, two PV matmuls.
  1/Z = stock DVE reciprocal on PSUM rows 64:128 (reciprocal_approx_fast is
  broken on this hardware), DVE multiply -> OT bf16.
  out[n,co] = sum_ci OT[ci,n] wpT[ci,co]; + bp via DVE add (replicated row).

Schedule notes (what the trace iterations taught us):
  - DMA: only use APs whose inner contiguous run is large. Each [128, 768]
    weight chunk of a (k p) n view is one linear 393KB region; the p-major
    single-DMA variant (768B segments) runs at ~2 GB/s/engine and starves
    everything.
  - ~8.5 us of throwaway warm-up matmuls at t=0 keep the HAM activity
    monitor at K=8/8 (2.4 GHz) while the weights load.
  - Block hp runs head-pair hp's 8-chunk attention with head-pair hp+1's
    Q/K projections interleaved at chunks 1/3/5 (V projection inside block
    0), so the PE never idles long enough to re-throttle.
  - Projection PSUM->SBUF casts run on the SCALAR engine: they land in the
    exp FIFO right where the PSUM pool rotation needs them, and keep the
    DVE free for the reciprocals (GpSimd cannot read PSUM; a DVE cast
    behind a 3.4 us reciprocal stalled the PE a full 7 us per block).
  - Output projection runs in two 2-bank PSUM waves, ci=5 last, so only
    the last pair's normalize sits on the critical path.
"""

import sys

for _p in ("/opt/trn_rl_repo",):
    if _p not in sys.path:
        sys.path.insert(0, _p)

import numpy as np
from contextlib import ExitStack

import concourse.bass as bass
import concourse.mybir as mybir
import concourse.tile as tile
from concourse import bacc

F32 = mybir.dt.float32
BF16 = mybir.dt.bfloat16

B = 4
N = 1024
M = 1024
C = 768
H = 12
D = 64
NSH = 512            # query rows per core
CK = C // 128        # 6 chunks of the feature dim
MK = M // 128        # 8 chunks of the key dim
HP = H // 2          # 6 head pairs (one KT/QT co-chunk each)
SCALE = D ** -0.5
N_CORES = 8
N_WARM = 28          # warm-up matmuls to keep HAM at 8/8 during loads


def build_bass():
    nc = bacc.Bacc("TRN2", target_bir_lowering=False, debug=False)

    xT = nc.dram_tensor("xT", [C, NSH], BF16, kind="ExternalInput").ap()
    yT = nc.dram_tensor("yT", [C, M], BF16, kind="ExternalInput").ap()
    ywr = nc.dram_tensor("ywr", [1, M], BF16, kind="ExternalInput").ap()
    wqT = nc.dram_tensor("wqT", [C, C], BF16, kind="ExternalInput").ap()
    wkT = nc.dram_tensor("wkT", [C, C], BF16, kind="ExternalInput").ap()
    wvT = nc.dram_tensor("wvT", [C, C], BF16, kind="ExternalInput").ap()
    wpT = nc.dram_tensor("wpT", [C, C], BF16, kind="ExternalInput").ap()
    bpf = nc.dram_tensor("bpf", [1, C], F32, kind="ExternalInput").ap()
    out = nc.dram_tensor("out", [NSH, C], F32, kind="ExternalOutput").ap()

    # k-major chunk views: each [128, x] chunk is one contiguous DRAM region
    wq_c = wqT.rearrange("(k p) n -> k p n", p=128)
    wk_c = wkT.rearrange("(k p) n -> k p n", p=128)
    wv_c = wvT.rearrange("(k p) n -> k p n", p=128)
    wp_c = wpT.rearrange("(k p) n -> k p n", p=128)
    xT_c = xT.rearrange("(k p) n -> k p n", p=128)
    yT_c = yT.rearrange("(k p) n -> k p n", p=128)
    out_c = out.rearrange("(k p) n -> k p n", p=128)

    with tile.TileContext(nc) as tc, ExitStack() as ctx:
        wpool = ctx.enter_context(tc.tile_pool(name="w", bufs=4))
        cpool = ctx.enter_context(tc.tile_pool(name="const", bufs=1))
        qpool = ctx.enter_context(tc.tile_pool(name="qt", bufs=3))
        kpool = ctx.enter_context(tc.tile_pool(name="kt", bufs=3))
        vpool = ctx.enter_context(tc.tile_pool(name="vs", bufs=MK))
        opool = ctx.enter_context(tc.tile_pool(name="ot", bufs=CK))
        epool = ctx.enter_context(tc.tile_pool(name="es", bufs=3))
        outpool = ctx.enter_context(tc.tile_pool(name="outs", bufs=2))
        zpool = ctx.enter_context(tc.tile_pool(name="z", bufs=4))
        ppool = ctx.enter_context(tc.tile_pool(name="pp", bufs=2, space="PSUM"))
        oppool = ctx.enter_context(tc.tile_pool(name="op", bufs=4, space="PSUM"))

        # ---- PE warm-up: throwaway matmuls with no DMA dependency ----
        wrm = cpool.tile([128, 512], BF16, tag="wrm")
        nc.gpsimd.memset(wrm, 0.0)
        ones = cpool.tile([1, C], BF16, tag="ones")
        nc.gpsimd.memset(ones, 1.0)
        wps = ppool.tile([128, 1024], F32, tag="pp", name="warmps")
        for i in range(N_WARM):
            nc.tensor.matmul(
                wps[:, 0:512], wrm[:, 0:128], wrm,
                start=(i == 0), stop=(i == N_WARM - 1),
            )

        # ---- input loads: contiguous per-chunk DMAs, 2 HWDGE rings ----
        # scalar ring: wq x6 (Q proj gate), yT x6, wp x6
        # sync ring:   xt x6, yw row, wk x6, wv x6, bp replicate
        wq = wpool.tile([128, CK, C], BF16, tag="w", name="wq")
        xt = cpool.tile([128, CK, NSH], BF16, tag="xt")
        for i in range(CK):
            nc.scalar.dma_start(out=wq[:, i, :], in_=wq_c[i])
            nc.sync.dma_start(out=xt[:, i, :], in_=xT_c[i])
        yw_s = cpool.tile([1, M], BF16, tag="yws")
        nc.sync.dma_start(out=yw_s, in_=ywr)
        yt = cpool.tile([128, CK, M], BF16, tag="yt")
        wk = wpool.tile([128, CK, C], BF16, tag="w", name="wk")
        for i in range(CK):
            nc.scalar.dma_start(out=yt[:, i, :], in_=yT_c[i])
            nc.sync.dma_start(out=wk[:, i, :], in_=wk_c[i])
        wv = wpool.tile([128, CK, C], BF16, tag="w", name="wv")
        for i in range(CK):
            nc.sync.dma_start(out=wv[:, i, :], in_=wv_c[i])
        wp = wpool.tile([128, CK, C], BF16, tag="w", name="wp")
        for i in range(CK):
            nc.scalar.dma_start(out=wp[:, i, :], in_=wp_c[i])
        bpb = cpool.tile([128, C], F32, tag="bpb")
        nc.sync.dma_start(
            out=bpb,
            in_=bass.AP(tensor=bpf.tensor, offset=0, ap=[[0, 128], [1, C]]),
        )
        # preload the ACT exp table off the critical path
        warm = cpool.tile([1, 8], F32, tag="warm")
        nc.scalar.activation(
            warm, bpb[0:1, 0:8], mybir.ActivationFunctionType.Exp, scale=SCALE
        )

        def qproj_mm(co):
            ps = ppool.tile([128, 1024], F32, tag="pp")
            for ci in range(CK):
                nc.tensor.matmul(
                    ps[:, 0:512],
                    wq[:, ci, co * 128:(co + 1) * 128],
                    xt[:, ci, :],
                    start=(ci == 0),
                    stop=(ci == CK - 1),
                )
            t = qpool.tile([128, NSH], BF16, tag="qt")
            nc.scalar.copy(t, ps[:, 0:512])
            return t

        def kproj_mm(co, t, mh):
            ps = ppool.tile([128, 1024], F32, tag="pp")
            sl = slice(mh * 512, (mh + 1) * 512)
            for ci in range(CK):
                nc.tensor.matmul(
                    ps[:, 0:512],
                    wk[:, ci, co * 128:(co + 1) * 128],
                    yt[:, ci, sl],
                    start=(ci == 0),
                    stop=False,
                )
            # += ones-row^T x yw-row: the additive key bias, rank-1
            nc.tensor.matmul(
                ps[:, 0:512],
                ones[:, co * 128:(co + 1) * 128],
                yw_s[:, sl],
                start=False,
                stop=True,
            )
            nc.scalar.copy(t[:, sl], ps[:, 0:512])

        def vproj(mc):
            t = vpool.tile([128, H, 128], BF16, tag="vs")
            nc.gpsimd.memset(t[:, :, 64:128], 1.0)
            ps = ppool.tile([128, 1024], F32, tag="pp")
            for nh in range(2):
                sl = slice(nh * 512, nh * 512 + 384)
                for ci in range(CK):
                    nc.tensor.matmul(
                        ps[:, sl],
                        yt[:, ci, mc * 128:(mc + 1) * 128],
                        wv[:, ci, nh * 384:(nh + 1) * 384],
                        start=(ci == 0),
                        stop=(ci == CK - 1),
                    )
            for nh in range(2):
                src = ps[:, nh * 512:nh * 512 + 384].rearrange(
                    "p (h e) -> p h e", e=64
                )
                nc.vector.tensor_copy(t[:, nh * 6:(nh + 1) * 6, 0:64], src)
            return t

        vt = [None] * MK
        ot = [None] * HP
        qt = [None] * HP
        kt = [None] * HP

        def attn_block(hp, build_v, build_next):
            """Head-pair hp's attention; next pair's projections (and, for
            hp==0, the V projection) sliced into the chunk loop.  The PV
            pair for chunk mc is emitted during chunk mc+1 so the PE never
            sits behind the in-flight exp."""
            h0, h1 = 2 * hp, 2 * hp + 1
            qtile, ktile = qt[hp], kt[hp]
            op0 = oppool.tile([128, 512], F32, tag="op", name=f"op{h0}")
            op1 = oppool.tile([128, 512], F32, tag="op", name=f"op{h1}")
            nxt = hp + 1
            ess = [None] * MK

            def pv(mc):
                nc.tensor.matmul(
                    op0, vt[mc][:, h0, :], ess[mc][:, 0:512],
                    start=(mc == 0), stop=(mc == MK - 1),
                )
                nc.tensor.matmul(
                    op1, vt[mc][:, h1, :], ess[mc][:, 512:1024],
                    start=(mc == 0), stop=(mc == MK - 1),
                )

            def proj_slice(slot):
                if not build_next:
                    return
                if slot == 1:
                    qt[nxt] = qproj_mm(nxt)
                elif slot == 3:
                    kt[nxt] = kpool.tile(
                        [128, M], BF16, tag="kt", name=f"kt{nxt}"
                    )
                    kproj_mm(nxt, kt[nxt], 0)
                elif slot == 5:
                    kproj_mm(nxt, kt[nxt], 1)

            for mc in range(MK):
                if build_v:
                    vt[mc] = vproj(mc)
                sp = ppool.tile([128, 1024], F32, tag="pp")
                nc.tensor.matmul(
                    sp[:, 0:512],
                    ktile[0:64, mc * 128:(mc + 1) * 128],
                    qtile[0:64, :],
                    start=True,
                    stop=True,
                )
                nc.tensor.matmul(
                    sp[:, 512:1024],
                    ktile[64:128, mc * 128:(mc + 1) * 128],
                    qtile[64:128, :],
                    start=True,
                    stop=True,
                )
                es = epool.tile([128, 1024], BF16, tag="es")
                nc.scalar.activation(
                    es, sp, mybir.ActivationFunctionType.Exp, scale=SCALE
                )
                ess[mc] = es
                if mc >= 1:
                    pv(mc - 1)
                proj_slice(mc)
            pv(MK - 1)
            return op0, op1

        def normalize(hp, op0, op1):
            t = opool.tile([128, NSH], BF16, tag="ot", name=f"ot{hp}")
            for j, op in enumerate((op0, op1)):
                zr = zpool.tile([64, 512], F32, tag="z")
                nc.vector.reciprocal(zr, op[64:128, :])
                nc.vector.tensor_tensor(
                    t[j * 64:(j + 1) * 64, :], op[0:64, :], zr,
                    mybir.AluOpType.mult,
                )
            ot[hp] = t

        qt[0] = qproj_mm(0)
        kt[0] = kpool.tile([128, M], BF16, tag="kt", name="kt0")
        kproj_mm(0, kt[0], 0)
        kproj_mm(0, kt[0], 1)
        prev = None
        for hp in range(HP):
            if prev is not None:
                normalize(hp - 1, *prev)
            prev = attn_block(
                hp, build_v=(hp == 0), build_next=(hp < HP - 1)
            )
        normalize(HP - 1, *prev)

        # ---- output projection + bias: two 2-bank waves.  ci=5 runs last
        # as two packed K=64 matmuls so wave A needs only ot[5][0:64]
        # (ready after the first reciprocal); throwaway warm matmuls keep
        # the HAM clock at 8/8 while the last normalize drains. ----
        for wave in range(2):
            pss = []
            for n4 in (2 * wave, 2 * wave + 1):
                ps = ppool.tile([128, 1024], F32, tag="pp")
                pss.append(ps)
                for nh in range(2):
                    sl = slice(nh * 512, nh * 512 + 384)
                    for ci in range(CK - 1):
                        nc.tensor.matmul(
                            ps[:, sl],
                            ot[ci][:, n4 * 128:(n4 + 1) * 128],
                            wp[:, ci, nh * 384:(nh + 1) * 384],
                            start=(ci == 0),
                            stop=False,
                        )
            if wave == 0:
                wop = oppool.tile([128, 512], F32, tag="op", name="tailwarm")
                for i in range(16):
                    nc.tensor.matmul(
                        wop, wrm[:, 0:128], wrm,
                        start=(i == 0), stop=(i == 15),
                    )
            for i, n4 in enumerate((2 * wave, 2 * wave + 1)):
                ps = pss[i]
                for nh in range(2):
                    sl = slice(nh * 512, nh * 512 + 384)
                    nc.tensor.matmul(
                        ps[:, sl],
                        ot[CK - 1][:, n4 * 128:(n4 + 1) * 128],
                        wp[:, CK - 1, nh * 384:(nh + 1) * 384],
                        start=False,
                        stop=True,
                    )
                outs = outpool.tile([128, C], F32, tag="outs")
                for nh in range(2):
                    nc.vector.tensor_tensor(
                        outs[:, nh * 384:(nh + 1) * 384],
                        ps[:, nh * 512:nh * 512 + 384],
                        bpb[:, nh * 384:(nh + 1) * 384],
                        mybir.AluOpType.add,
                    )
                nc.sync.dma_start(out=out_c[n4], in_=outs)

    if not nc.is_finalized():
        nc.finalize()
    return nc


_NC_CACHE = None


def _get_nc():
    global _NC_CACHE
    if _NC_CACHE is None:
        _NC_CACHE = build_bass()
    return _NC_CACHE


def make_in_maps(x, y, yw, Wq, Wk, Wv, Wp, bp):
    import ml_dtypes

    bf = ml_dtypes.bfloat16
    x = np.asarray(x, np.float32)
    y = np.asarray(y, np.float32)
    yw = np.asarray(yw, np.float32)
    wqT = np.ascontiguousarray(np.asarray(Wq, np.float32).T).astype(bf)
    wkT = np.ascontiguousarray(np.asarray(Wk, np.float32).T).astype(bf)
    wvT = np.ascontiguousarray(np.asarray(Wv, np.float32).T).astype(bf)
    wpT = np.ascontiguousarray(np.asarray(Wp, np.float32).T).astype(bf)
    bpf = np.asarray(bp, np.float32).reshape(1, C)

    in_maps = []
    for c in range(N_CORES):
        b, half = divmod(c, 2)
        n0 = half * NSH
        in_maps.append(
            {
                "xT": np.ascontiguousarray(x[b, n0:n0 + NSH, :].T).astype(bf),
                "yT": np.ascontiguousarray(y[b].T).astype(bf),
                "ywr": np.ascontiguousarray(yw[b].reshape(1, M)).astype(bf),
                "wqT": wqT,
                "wkT": wkT,
                "wvT": wvT,
                "wpT": wpT,
                "bpf": bpf,
            }
        )
    return in_maps


def run(inputs, trace=False):
    """Returns (full_output, BassKernelResults)."""
    from concourse.bass_utils import run_bass_kernel_spmd

    nc = _get_nc()
    in_maps = make_in_maps(**inputs)
    res = run_bass_kernel_spmd(
        nc, in_maps, list(range(N_CORES)), trace=trace
    )
    full = np.empty((B, N, C), dtype=np.float32)
    for c in range(N_CORES):
        b, half = divmod(c, 2)
        n0 = half * NSH
        full[b, n0:n0 + NSH, :] = res.results[c]["out"]
    return full, res


def kernel(**inputs):
    full, _ = run(inputs, trace=False)
    return full


# revision 18
# speedup vs baseline: 2.0718x; 1.0005x over previous
"""Trainium2 Bass kernel for nn_CrossAttention (B=4, N=M=1024, C=768, H=12, D=64).

Sharding: pure data-parallel over 8 cores. Core c handles batch b = c // 2 and
query rows [512*(c%2), 512*(c%2)+512). Each core computes K/V for its batch
(duplicated across the 2 cores sharing a batch) so no collectives are needed.

All-bf16 datapath (fp32 PSUM accumulation); bf16 streams the PE at 1 cycle/row
and avoids the fp32r power throttle. Host-side layout:
  xT  [768, 512]   = x[b, n0:n0+512, :].T   (c-major for Q projection)
  yT  [768, 1024]  = y[b].T                 (c-major for K/V projection)
  w*T [768, 768]   = W.T                    (c-major weights)
  ywr [1, 1024]    = yw[b] row (bf16), bp fp32 row (DMA-replicated to 128
                     partitions for the DVE bias add)

Device dataflow (all matmuls bf16 x bf16 -> fp32 PSUM):
  QT[co,n] = sum_c wqT[c,co] xT[c,n]
  KT[co,m] = sum_c wkT[c,co] yT[c,m] + ones-row x ywr (rank-1 bias matmul)
  V[m,cv]  = sum_c yT[c,m] wvT[c,cv]  in a [128, 12, 128] per-chunk layout
             whose cols 64:128 are memset to 1 so the PV matmul's PSUM rows
             64:128 accumulate Z replicated 64x (softmax denominator, free)
  per head PAIR: two K=64 S-matmuls at PE array tile positions (0,0)/(64,0)
  (they execute concurrently on disjoint sub-arrays) into one [128,1024]
  PSUM tile, ONE exp over [128,1024] -> bf16, two PV matmuls.
  1/Z = stock DVE reciprocal on PSUM rows 64:128 (reciprocal_approx_fast is
  broken on this hardware), DVE multiply -> OT bf16.
  out[n,co] = sum_ci OT[ci,n] wpT[ci,co]; + bp via DVE add (replicated row).

Schedule notes (what the trace iterations taught us):
  - DMA: only use APs whose inner contiguous run is large. Each [128, 768]
    weight chunk of a (k p) n view is one linear 393KB region; the p-major
    single-DMA variant (768B segments) runs at ~2 GB/s/engine and starves
    everything.
  - ~8.5 us of throwaway warm-up matmuls at t=0 keep the HAM activity
    monitor at K=8/8 (2.4 GHz) while the weights load.
  - Block hp runs head-pair hp's 8-chunk attention with head-pair hp+1's
    Q/K projections interleaved at chunks 1/3/5 (V projection inside block
    0), so the PE never idles long enough to re-throttle.
  - Projection PSUM->SBUF casts run on the SCALAR engine: they land in the
    exp FIFO right where the PSUM pool rotation needs them, and keep the
    DVE free for the reciprocals (GpSimd cannot read PSUM; a DVE cast
    behind a 3.4 us reciprocal stalled the PE a full 7 us per block).
  - Output projection runs in two 2-bank PSUM waves, ci=5 last, so only
    the last pair's normalize sits on the critical path.
"""

import sys

for _p in ("/opt/trn_rl_repo",):
    if _p not in sys.path:
        sys.path.insert(0, _p)

import numpy as np
from contextlib import ExitStack

import concourse.bass as bass
import concourse.mybir as mybir
import concourse.tile as tile
from concourse import bacc

F32 = mybir.dt.float32
BF16 = mybir.dt.bfloat16

B = 4
N = 1024
M = 1024
C = 768
H = 12
D = 64
NSH = 512            # query rows per core
CK = C // 128        # 6 chunks of the feature dim
MK = M // 128        # 8 chunks of the key dim
HP = H // 2          # 6 head pairs (one KT/QT co-chunk each)
SCALE = D ** -0.5
N_CORES = 8
N_WARM = 28          # warm-up matmuls to keep HAM at 8/8 during loads


def build_bass():
    nc = bacc.Bacc("TRN2", target_bir_lowering=False, debug=False)

    xT = nc.dram_tensor("xT", [C, NSH], BF16, kind="ExternalInput").ap()
    yT = nc.dram_tensor("yT", [C, M], BF16, kind="ExternalInput").ap()
    ywr = nc.dram_tensor("ywr", [1, M], BF16, kind="ExternalInput").ap()
    wqT = nc.dram_tensor("wqT", [C, C], BF16, kind="ExternalInput").ap()
    wkT = nc.dram_tensor("wkT", [C, C], BF16, kind="ExternalInput").ap()
    wvT = nc.dram_tensor("wvT", [C, C], BF16, kind="ExternalInput").ap()
    wpT = nc.dram_tensor("wpT", [C, C], BF16, kind="ExternalInput").ap()
    bpf = nc.dram_tensor("bpf", [1, C], F32, kind="ExternalInput").ap()
    out = nc.dram_tensor("out", [NSH, C], F32, kind="ExternalOutput").ap()

    # k-major chunk views: each [128, x] chunk is one contiguous DRAM region
    wq_c = wqT.rearrange("(k p) n -> k p n", p=128)
    wk_c = wkT.rearrange("(k p) n -> k p n", p=128)
    wv_c = wvT.rearrange("(k p) n -> k p n", p=128)
    wp_c = wpT.rearrange("(k p) n -> k p n", p=128)
    xT_c = xT.rearrange("(k p) n -> k p n", p=128)
    yT_c = yT.rearrange("(k p) n -> k p n", p=128)
    out_c = out.rearrange("(k p) n -> k p n", p=128)

    with tile.TileContext(nc) as tc, ExitStack() as ctx:
        wpool = ctx.enter_context(tc.tile_pool(name="w", bufs=4))
        cpool = ctx.enter_context(tc.tile_pool(name="const", bufs=1))
        qpool = ctx.enter_context(tc.tile_pool(name="qt", bufs=3))
        kpool = ctx.enter_context(tc.tile_pool(name="kt", bufs=3))
        vpool = ctx.enter_context(tc.tile_pool(name="vs", bufs=MK))
        opool = ctx.enter_context(tc.tile_pool(name="ot", bufs=CK))
        epool = ctx.enter_context(tc.tile_pool(name="es", bufs=3))
        outpool = ctx.enter_context(tc.tile_pool(name="outs", bufs=2))
        zpool = ctx.enter_context(tc.tile_pool(name="z", bufs=4))
        ppool = ctx.enter_context(tc.tile_pool(name="pp", bufs=2, space="PSUM"))
        oppool = ctx.enter_context(tc.tile_pool(name="op", bufs=4, space="PSUM"))

        # ---- PE warm-up: throwaway matmuls with no DMA dependency ----
        wrm = cpool.tile([128, 512], BF16, tag="wrm")
        nc.gpsimd.memset(wrm, 0.0)
        ones = cpool.tile([1, C], BF16, tag="ones")
        nc.gpsimd.memset(ones, 1.0)
        wps = ppool.tile([128, 1024], F32, tag="pp", name="warmps")
        for i in range(N_WARM):
            nc.tensor.matmul(
                wps[:, 0:512], wrm[:, 0:128], wrm,
                start=(i == 0), stop=(i == N_WARM - 1),
            )

        # ---- input loads: contiguous per-chunk DMAs, 2 HWDGE rings ----
        # scalar ring: wq x6 (Q proj gate), yT x6, wp x6
        # sync ring:   xt x6, yw row, wk x6, wv x6, bp replicate
        wq = wpool.tile([128, CK, C], BF16, tag="w", name="wq")
        xt = cpool.tile([128, CK, NSH], BF16, tag="xt")
        for i in range(CK):
            nc.scalar.dma_start(out=wq[:, i, :], in_=wq_c[i])
            nc.sync.dma_start(out=xt[:, i, :], in_=xT_c[i])
        yw_s = cpool.tile([1, M], BF16, tag="yws")
        nc.sync.dma_start(out=yw_s, in_=ywr)
        yt = cpool.tile([128, CK, M], BF16, tag="yt")
        wk = wpool.tile([128, CK, C], BF16, tag="w", name="wk")
        for i in range(CK):
            nc.scalar.dma_start(out=yt[:, i, :], in_=yT_c[i])
            nc.sync.dma_start(out=wk[:, i, :], in_=wk_c[i])
        wv = wpool.tile([128, CK, C], BF16, tag="w", name="wv")
        for i in range(CK):
            nc.sync.dma_start(out=wv[:, i, :], in_=wv_c[i])
        # wp is not needed until the output projection: load it via the
        # gpsimd SWDGE ring so it costs neither HWDGE ring any bandwidth
        # and adds no issue time on the ACT queue.
        wp = wpool.tile([128, CK, C], BF16, tag="w", name="wp")
        for i in range(CK):
            nc.gpsimd.dma_start(out=wp[:, i, :], in_=wp_c[i])
        bpb = cpool.tile([128, C], F32, tag="bpb")
        nc.sync.dma_start(
            out=bpb,
            in_=bass.AP(tensor=bpf.tensor, offset=0, ap=[[0, 128], [1, C]]),
        )
        # preload the ACT exp table from the memset ones tile: zero DMA
        # dependency, so the table load cannot block the projection casts
        # queued behind it on the ACT engine.
        warm = cpool.tile([1, 8], BF16, tag="warm")
        nc.scalar.activation(
            warm, ones[0:1, 0:8], mybir.ActivationFunctionType.Exp,
            scale=SCALE,
        )

        def qproj_mm(co):
            ps = ppool.tile([128, 1024], F32, tag="pp")
            for ci in range(CK):
                nc.tensor.matmul(
                    ps[:, 0:512],
                    wq[:, ci, co * 128:(co + 1) * 128],
                    xt[:, ci, :],
                    start=(ci == 0),
                    stop=(ci == CK - 1),
                )
            t = qpool.tile([128, NSH], BF16, tag="qt")
            nc.scalar.copy(t, ps[:, 0:512])
            return t

        def kproj_mm(co, t, mh):
            ps = ppool.tile([128, 1024], F32, tag="pp")
            sl = slice(mh * 512, (mh + 1) * 512)
            for ci in range(CK):
                nc.tensor.matmul(
                    ps[:, 0:512],
                    wk[:, ci, co * 128:(co + 1) * 128],
                    yt[:, ci, sl],
                    start=(ci == 0),
                    stop=False,
                )
            # += ones-row^T x yw-row: the additive key bias, rank-1
            nc.tensor.matmul(
                ps[:, 0:512],
                ones[:, co * 128:(co + 1) * 128],
                yw_s[:, sl],
                start=False,
                stop=True,
            )
            nc.scalar.copy(t[:, sl], ps[:, 0:512])

        def vproj(mc):
            t = vpool.tile([128, H, 128], BF16, tag="vs")
            nc.gpsimd.memset(t[:, :, 64:128], 1.0)
            ps = ppool.tile([128, 1024], F32, tag="pp")
            for nh in range(2):
                sl = slice(nh * 512, nh * 512 + 384)
                for ci in range(CK):
                    nc.tensor.matmul(
                        ps[:, sl],
                        yt[:, ci, mc * 128:(mc + 1) * 128],
                        wv[:, ci, nh * 384:(nh + 1) * 384],
                        start=(ci == 0),
                        stop=(ci == CK - 1),
                    )
            for nh in range(2):
                src = ps[:, nh * 512:nh * 512 + 384].rearrange(
                    "p (h e) -> p h e", e=64
                )
                nc.vector.tensor_copy(t[:, nh * 6:(nh + 1) * 6, 0:64], src)
            return t

        vt = [None] * MK
        ot = [None] * HP
        qt = [None] * HP
        kt = [None] * HP

        def attn_block(hp, build_v, build_next):
            """Head-pair hp's attention; next pair's projections (and, for
            hp==0, the V projection) sliced into the chunk loop.  The PV
            pair for chunk mc is emitted during chunk mc+1 so the PE never
            sits behind the in-flight exp."""
            h0, h1 = 2 * hp, 2 * hp + 1
            qtile, ktile = qt[hp], kt[hp]
            op0 = oppool.tile([128, 512], F32, tag="op", name=f"op{h0}")
            op1 = oppool.tile([128, 512], F32, tag="op", name=f"op{h1}")
            nxt = hp + 1
            ess = [None] * MK

            def pv(mc):
                nc.tensor.matmul(
                    op0, vt[mc][:, h0, :], ess[mc][:, 0:512],
                    start=(mc == 0), stop=(mc == MK - 1),
                )
                nc.tensor.matmul(
                    op1, vt[mc][:, h1, :], ess[mc][:, 512:1024],
                    start=(mc == 0), stop=(mc == MK - 1),
                )

            def proj_slice(slot):
                if not build_next:
                    return
                if slot == 1:
                    qt[nxt] = qproj_mm(nxt)
                elif slot == 3:
                    kt[nxt] = kpool.tile(
                        [128, M], BF16, tag="kt", name=f"kt{nxt}"
                    )
                    kproj_mm(nxt, kt[nxt], 0)
                elif slot == 5:
                    kproj_mm(nxt, kt[nxt], 1)

            for mc in range(MK):
                sp = ppool.tile([128, 1024], F32, tag="pp")
                nc.tensor.matmul(
                    sp[:, 0:512],
                    ktile[0:64, mc * 128:(mc + 1) * 128],
                    qtile[0:64, :],
                    start=True,
                    stop=True,
                )
                nc.tensor.matmul(
                    sp[:, 512:1024],
                    ktile[64:128, mc * 128:(mc + 1) * 128],
                    qtile[64:128, :],
                    start=True,
                    stop=True,
                )
                es = epool.tile([128, 1024], BF16, tag="es")
                nc.scalar.activation(
                    es, sp, mybir.ActivationFunctionType.Exp, scale=SCALE
                )
                ess[mc] = es
                if build_v:
                    vt[mc] = vproj(mc)
                if mc >= 1:
                    pv(mc - 1)
                proj_slice(mc)
            pv(MK - 1)
            return op0, op1

        def normalize(hp, op0, op1):
            t = opool.tile([128, NSH], BF16, tag="ot", name=f"ot{hp}")
            for j, op in enumerate((op0, op1)):
                zr = zpool.tile([64, 512], F32, tag="z")
                nc.vector.reciprocal(zr, op[64:128, :])
                nc.vector.tensor_tensor(
                    t[j * 64:(j + 1) * 64, :], op[0:64, :], zr,
                    mybir.AluOpType.mult,
                )
            ot[hp] = t

        qt[0] = qproj_mm(0)
        kt[0] = kpool.tile([128, M], BF16, tag="kt", name="kt0")
        kproj_mm(0, kt[0], 0)
        kproj_mm(0, kt[0], 1)
        prev = None
        for hp in range(HP):
            if prev is not None:
                normalize(hp - 1, *prev)
            prev = attn_block(
                hp, build_v=(hp == 0), build_next=(hp < HP - 1)
            )
        normalize(HP - 1, *prev)

        # ---- output projection + bias: two 2-bank waves.  ci=5 runs last
        # as two packed K=64 matmuls so wave A needs only ot[5][0:64]
        # (ready after the first reciprocal); throwaway warm matmuls keep
        # the HAM clock at 8/8 while the last normalize drains. ----
        for wave in range(2):
            pss = []
            for n4 in (2 * wave, 2 * wave + 1):
                ps = ppool.tile([128, 1024], F32, tag="pp")
                pss.append(ps)
                for nh in range(2):
                    sl = slice(nh * 512, nh * 512 + 384)
                    for ci in range(CK - 1):
                        nc.tensor.matmul(
                            ps[:, sl],
                            ot[ci][:, n4 * 128:(n4 + 1) * 128],
                            wp[:, ci, nh * 384:(nh + 1) * 384],
                            start=(ci == 0),
                            stop=False,
                        )
            if wave == 0:
                wop = oppool.tile([128, 512], F32, tag="op", name="tailwarm")
                for i in range(16):
                    nc.tensor.matmul(
                        wop, wrm[:, 0:128], wrm,
                        start=(i == 0), stop=(i == 15),
                    )
            for i, n4 in enumerate((2 * wave, 2 * wave + 1)):
                ps = pss[i]
                for nh in range(2):
                    sl = slice(nh * 512, nh * 512 + 384)
                    nc.tensor.matmul(
                        ps[:, sl],
                        ot[CK - 1][:, n4 * 128:(n4 + 1) * 128],
                        wp[:, CK - 1, nh * 384:(nh + 1) * 384],
                        start=False,
                        stop=True,
                    )
                outs = outpool.tile([128, C], F32, tag="outs")
                for nh in range(2):
                    nc.vector.tensor_tensor(
                        outs[:, nh * 384:(nh + 1) * 384],
                        ps[:, nh * 512:nh * 512 + 384],
                        bpb[:, nh * 384:(nh + 1) * 384],
                        mybir.AluOpType.add,
                    )
                nc.sync.dma_start(out=out_c[n4], in_=outs)

    if not nc.is_finalized():
        nc.finalize()
    return nc


_NC_CACHE = None


def _get_nc():
    global _NC_CACHE
    if _NC_CACHE is None:
        _NC_CACHE = build_bass()
    return _NC_CACHE


def make_in_maps(x, y, yw, Wq, Wk, Wv, Wp, bp):
    import ml_dtypes

    bf = ml_dtypes.bfloat16
    x = np.asarray(x, np.float32)
    y = np.asarray(y, np.float32)
    yw = np.asarray(yw, np.float32)
    wqT = np.ascontiguousarray(np.asarray(Wq, np.float32).T).astype(bf)
    wkT = np.ascontiguousarray(np.asarray(Wk, np.float32).T).astype(bf)
    wvT = np.ascontiguousarray(np.asarray(Wv, np.float32).T).astype(bf)
    wpT = np.ascontiguousarray(np.asarray(Wp, np.float32).T).astype(bf)
    bpf = np.asarray(bp, np.float32).reshape(1, C)

    in_maps = []
    for c in range(N_CORES):
        b, half = divmod(c, 2)
        n0 = half * NSH
        in_maps.append(
            {
                "xT": np.ascontiguousarray(x[b, n0:n0 + NSH, :].T).astype(bf),
                "yT": np.ascontiguousarray(y[b].T).astype(bf),
                "ywr": np.ascontiguousarray(yw[b].reshape(1, M)).astype(bf),
                "wqT": wqT,
                "wkT": wkT,
                "wvT": wvT,
                "wpT": wpT,
                "bpf": bpf,
            }
        )
    return in_maps


def run(inputs, trace=False):
    """Returns (full_output, BassKernelResults)."""
    from concourse.bass_utils import run_bass_kernel_spmd

    nc = _get_nc()
    in_maps = make_in_maps(**inputs)
    res = run_bass_kernel_spmd(
        nc, in_maps, list(range(N_CORES)), trace=trace
    )
    full = np.empty((B, N, C), dtype=np.float32)
    for c in range(N_CORES):
        b, half = divmod(c, 2)
        n0 = half * NSH
        full[b, n0:n0 + NSH, :] = res.results[c]["out"]
    return full, res


def kernel(**inputs):
    full, _ = run(inputs, trace=False)
    return full


# revision 19
# speedup vs baseline: 2.1044x; 1.0158x over previous
"""Trainium2 Bass kernel for nn_CrossAttention (B=4, N=M=1024, C=768, H=12, D=64).

Sharding: pure data-parallel over 8 cores. Core c handles batch b = c // 2 and
query rows [512*(c%2), 512*(c%2)+512). Each core computes K/V for its batch
(duplicated across the 2 cores sharing a batch) so no collectives are needed.

All-bf16 datapath (fp32 PSUM accumulation); bf16 streams the PE at 1 cycle/row
and avoids the fp32r power throttle. Host-side layout:
  xT  [768, 512]   = x[b, n0:n0+512, :].T   (c-major for Q projection)
  yT  [768, 1024]  = y[b].T                 (c-major for K/V projection)
  w*T [768, 768]   = W.T                    (c-major weights)
  ywr [1, 1024]    = yw[b] row (bf16), bp fp32 row (DMA-replicated to 128
                     partitions for the DVE bias add)

Device dataflow (all matmuls bf16 x bf16 -> fp32 PSUM):
  QT[co,n] = sum_c wqT[c,co] xT[c,n]
  KT[co,m] = sum_c wkT[c,co] yT[c,m] + ones-row x ywr (rank-1 bias matmul)
  V[m,cv]  = sum_c yT[c,m] wvT[c,cv]  in a [128, 12, 128] per-chunk layout
             whose cols 64:128 are memset to 1 so the PV matmul's PSUM rows
             64:128 accumulate Z replicated 64x (softmax denominator, free)
  per head PAIR: two K=64 S-matmuls at PE array tile positions (0,0)/(64,0)
  (they execute concurrently on disjoint sub-arrays) into one [128,1024]
  PSUM tile, ONE exp over [128,1024] -> bf16, two PV matmuls.
  1/Z = stock DVE reciprocal on PSUM rows 64:128 (reciprocal_approx_fast is
  broken on this hardware), DVE multiply -> OT bf16.
  out[n,co] = sum_ci OT[ci,n] wpT[ci,co]; + bp via DVE add (replicated row).

Schedule notes (what the trace iterations taught us):
  - DMA: only use APs whose inner contiguous run is large. Each [128, 768]
    weight chunk of a (k p) n view is one linear 393KB region; the p-major
    single-DMA variant (768B segments) runs at ~2 GB/s/engine and starves
    everything.
  - ~8.5 us of throwaway warm-up matmuls at t=0 keep the HAM activity
    monitor at K=8/8 (2.4 GHz) while the weights load.
  - Block hp runs head-pair hp's 8-chunk attention with head-pair hp+1's
    Q/K projections interleaved at chunks 1/3/5 (V projection inside block
    0), so the PE never idles long enough to re-throttle.
  - Projection PSUM->SBUF casts run on the SCALAR engine: they land in the
    exp FIFO right where the PSUM pool rotation needs them, and keep the
    DVE free for the reciprocals (GpSimd cannot read PSUM; a DVE cast
    behind a 3.4 us reciprocal stalled the PE a full 7 us per block).
  - Output projection runs in two 2-bank PSUM waves, ci=5 last, so only
    the last pair's normalize sits on the critical path.
"""

import sys

for _p in ("/opt/trn_rl_repo",):
    if _p not in sys.path:
        sys.path.insert(0, _p)

import numpy as np
from contextlib import ExitStack

import concourse.bass as bass
import concourse.mybir as mybir
import concourse.tile as tile
from concourse import bacc

F32 = mybir.dt.float32
BF16 = mybir.dt.bfloat16

B = 4
N = 1024
M = 1024
C = 768
H = 12
D = 64
NSH = 512            # query rows per core
CK = C // 128        # 6 chunks of the feature dim
MK = M // 128        # 8 chunks of the key dim
HP = H // 2          # 6 head pairs (one KT/QT co-chunk each)
SCALE = D ** -0.5
N_CORES = 8
N_WARM = 28          # warm-up matmuls to keep HAM at 8/8 during loads


def build_bass():
    nc = bacc.Bacc("TRN2", target_bir_lowering=False, debug=False)

    xT = nc.dram_tensor("xT", [C, NSH], BF16, kind="ExternalInput").ap()
    yT = nc.dram_tensor("yT", [C, M], BF16, kind="ExternalInput").ap()
    ywr = nc.dram_tensor("ywr", [1, M], BF16, kind="ExternalInput").ap()
    wqT = nc.dram_tensor("wqT", [C, C], BF16, kind="ExternalInput").ap()
    wkT = nc.dram_tensor("wkT", [C, C], BF16, kind="ExternalInput").ap()
    wvT = nc.dram_tensor("wvT", [C, C], BF16, kind="ExternalInput").ap()
    wpT = nc.dram_tensor("wpT", [C, C], BF16, kind="ExternalInput").ap()
    bpf = nc.dram_tensor("bpf", [1, C], F32, kind="ExternalInput").ap()
    out = nc.dram_tensor("out", [NSH, C], F32, kind="ExternalOutput").ap()

    # k-major chunk views: each [128, x] chunk is one contiguous DRAM region
    wq_c = wqT.rearrange("(k p) n -> k p n", p=128)
    wk_c = wkT.rearrange("(k p) n -> k p n", p=128)
    wv_c = wvT.rearrange("(k p) n -> k p n", p=128)
    wp_c = wpT.rearrange("(k p) n -> k p n", p=128)
    xT_c = xT.rearrange("(k p) n -> k p n", p=128)
    yT_c = yT.rearrange("(k p) n -> k p n", p=128)
    out_c = out.rearrange("(k p) n -> k p n", p=128)

    with tile.TileContext(nc) as tc, ExitStack() as ctx:
        wpool = ctx.enter_context(tc.tile_pool(name="w", bufs=4))
        cpool = ctx.enter_context(tc.tile_pool(name="const", bufs=1))
        qpool = ctx.enter_context(tc.tile_pool(name="qt", bufs=3))
        kpool = ctx.enter_context(tc.tile_pool(name="kt", bufs=3))
        vpool = ctx.enter_context(tc.tile_pool(name="vs", bufs=MK))
        opool = ctx.enter_context(tc.tile_pool(name="ot", bufs=CK))
        epool = ctx.enter_context(tc.tile_pool(name="es", bufs=3))
        outpool = ctx.enter_context(tc.tile_pool(name="outs", bufs=2))
        zpool = ctx.enter_context(tc.tile_pool(name="z", bufs=4))
        ppool = ctx.enter_context(tc.tile_pool(name="pp", bufs=2, space="PSUM"))
        oppool = ctx.enter_context(tc.tile_pool(name="op", bufs=4, space="PSUM"))

        # ---- PE warm-up: throwaway matmuls with no DMA dependency ----
        wrm = cpool.tile([128, 512], BF16, tag="wrm")
        nc.gpsimd.memset(wrm, 0.0)
        ones = cpool.tile([1, C], BF16, tag="ones")
        nc.gpsimd.memset(ones, 1.0)
        wps = ppool.tile([128, 1024], F32, tag="pp", name="warmps")
        for i in range(N_WARM):
            nc.tensor.matmul(
                wps[:, 0:512], wrm[:, 0:128], wrm,
                start=(i == 0), stop=(i == N_WARM - 1),
            )

        # ---- input loads: contiguous per-chunk DMAs, 2 HWDGE rings ----
        # scalar ring: wq x6 (Q proj gate), yT x6, wp x6
        # sync ring:   xt x6, yw row, wk x6, wv x6, bp replicate
        wq = wpool.tile([128, CK, C], BF16, tag="w", name="wq")
        xt = cpool.tile([128, CK, NSH], BF16, tag="xt")
        for i in range(CK):
            nc.scalar.dma_start(out=wq[:, i, :], in_=wq_c[i])
            nc.sync.dma_start(out=xt[:, i, :], in_=xT_c[i])
        yw_s = cpool.tile([1, M], BF16, tag="yws")
        nc.sync.dma_start(out=yw_s, in_=ywr)
        yt = cpool.tile([128, CK, M], BF16, tag="yt")
        wk = wpool.tile([128, CK, C], BF16, tag="w", name="wk")
        for i in range(CK):
            nc.scalar.dma_start(out=yt[:, i, :], in_=yT_c[i])
            nc.sync.dma_start(out=wk[:, i, :], in_=wk_c[i])
        wv = wpool.tile([128, CK, C], BF16, tag="w", name="wv")
        for i in range(CK):
            nc.sync.dma_start(out=wv[:, i, :], in_=wv_c[i])
        # wp is not needed until the output projection: load it via the
        # gpsimd SWDGE ring so it costs neither HWDGE ring any bandwidth
        # and adds no issue time on the ACT queue.
        wp = wpool.tile([128, CK, C], BF16, tag="w", name="wp")
        for i in range(CK):
            nc.gpsimd.dma_start(out=wp[:, i, :], in_=wp_c[i])
        bpb = cpool.tile([128, C], F32, tag="bpb")
        nc.sync.dma_start(
            out=bpb,
            in_=bass.AP(tensor=bpf.tensor, offset=0, ap=[[0, 128], [1, C]]),
        )
        # preload the ACT exp table from the memset ones tile: zero DMA
        # dependency, so the table load cannot block the projection casts
        # queued behind it on the ACT engine.
        warm = cpool.tile([1, 8], BF16, tag="warm")
        nc.scalar.activation(
            warm, ones[0:1, 0:8], mybir.ActivationFunctionType.Exp,
            scale=SCALE,
        )

        def qproj_mm(co):
            ps = ppool.tile([128, 1024], F32, tag="pp")
            for ci in range(CK):
                nc.tensor.matmul(
                    ps[:, 0:512],
                    wq[:, ci, co * 128:(co + 1) * 128],
                    xt[:, ci, :],
                    start=(ci == 0),
                    stop=(ci == CK - 1),
                )
            t = qpool.tile([128, NSH], BF16, tag="qt")
            nc.scalar.copy(t, ps[:, 0:512])
            return t

        def kproj_mm(co, t, mh):
            ps = ppool.tile([128, 1024], F32, tag="pp")
            sl = slice(mh * 512, (mh + 1) * 512)
            for ci in range(CK):
                nc.tensor.matmul(
                    ps[:, 0:512],
                    wk[:, ci, co * 128:(co + 1) * 128],
                    yt[:, ci, sl],
                    start=(ci == 0),
                    stop=False,
                )
            # += ones-row^T x yw-row: the additive key bias, rank-1
            nc.tensor.matmul(
                ps[:, 0:512],
                ones[:, co * 128:(co + 1) * 128],
                yw_s[:, sl],
                start=False,
                stop=True,
            )
            nc.scalar.copy(t[:, sl], ps[:, 0:512])

        def vproj(mc):
            t = vpool.tile([128, H, 128], BF16, tag="vs")
            nc.gpsimd.memset(t[:, :, 64:128], 1.0)
            ps = ppool.tile([128, 1024], F32, tag="pp")
            for nh in range(2):
                sl = slice(nh * 512, nh * 512 + 384)
                for ci in range(CK):
                    nc.tensor.matmul(
                        ps[:, sl],
                        yt[:, ci, mc * 128:(mc + 1) * 128],
                        wv[:, ci, nh * 384:(nh + 1) * 384],
                        start=(ci == 0),
                        stop=(ci == CK - 1),
                    )
            for nh in range(2):
                src = ps[:, nh * 512:nh * 512 + 384].rearrange(
                    "p (h e) -> p h e", e=64
                )
                nc.vector.tensor_copy(t[:, nh * 6:(nh + 1) * 6, 0:64], src)
            return t

        vt = [None] * MK
        ot = [None] * HP
        qt = [None] * HP
        kt = [None] * HP

        def attn_block(hp, build_v, build_next):
            """Head-pair hp's attention; next pair's projections (and, for
            hp==0, the V projection) sliced into the chunk loop.  The PV
            pair for chunk mc is emitted during chunk mc+1 so the PE never
            sits behind the in-flight exp."""
            h0, h1 = 2 * hp, 2 * hp + 1
            qtile, ktile = qt[hp], kt[hp]
            op0 = oppool.tile([128, 512], F32, tag="op", name=f"op{h0}")
            op1 = oppool.tile([128, 512], F32, tag="op", name=f"op{h1}")
            nxt = hp + 1
            ess = [None] * MK

            def pv(mc):
                nc.tensor.matmul(
                    op0, vt[mc][:, h0, :], ess[mc][:, 0:512],
                    start=(mc == 0), stop=(mc == MK - 1),
                )
                nc.tensor.matmul(
                    op1, vt[mc][:, h1, :], ess[mc][:, 512:1024],
                    start=(mc == 0), stop=(mc == MK - 1),
                )

            def proj_slice(slot):
                if not build_next:
                    return
                # each branch allocates ONE psum tile; the extra untouched
                # dummy alloc keeps the 2-buffer pp rotation parity so
                # S-pair(mc) reuses sp(mc-2) (pipeline depth 2), not the
                # tile freed by the previous chunk's exp.
                if slot == 1:
                    qt[nxt] = qproj_mm(nxt)
                    ppool.tile([128, 1024], F32, tag="pp", name=f"dq{nxt}")
                elif slot == 3:
                    kt[nxt] = kpool.tile(
                        [128, M], BF16, tag="kt", name=f"kt{nxt}"
                    )
                    kproj_mm(nxt, kt[nxt], 0)
                    ppool.tile([128, 1024], F32, tag="pp", name=f"da{nxt}")
                elif slot == 5:
                    kproj_mm(nxt, kt[nxt], 1)
                    ppool.tile([128, 1024], F32, tag="pp", name=f"db{nxt}")

            for mc in range(MK):
                sp = ppool.tile([128, 1024], F32, tag="pp")
                nc.tensor.matmul(
                    sp[:, 0:512],
                    ktile[0:64, mc * 128:(mc + 1) * 128],
                    qtile[0:64, :],
                    start=True,
                    stop=True,
                )
                nc.tensor.matmul(
                    sp[:, 512:1024],
                    ktile[64:128, mc * 128:(mc + 1) * 128],
                    qtile[64:128, :],
                    start=True,
                    stop=True,
                )
                es = epool.tile([128, 1024], BF16, tag="es")
                nc.scalar.activation(
                    es, sp, mybir.ActivationFunctionType.Exp, scale=SCALE
                )
                ess[mc] = es
                if build_v:
                    vt[mc] = vproj(mc)
                if mc >= 1:
                    pv(mc - 1)
                proj_slice(mc)
            pv(MK - 1)
            return op0, op1

        def normalize(hp, op0, op1):
            t = opool.tile([128, NSH], BF16, tag="ot", name=f"ot{hp}")
            for j, op in enumerate((op0, op1)):
                zr = zpool.tile([64, 512], F32, tag="z")
                nc.vector.reciprocal(zr, op[64:128, :])
                nc.vector.tensor_tensor(
                    t[j * 64:(j + 1) * 64, :], op[0:64, :], zr,
                    mybir.AluOpType.mult,
                )
            ot[hp] = t

        qt[0] = qproj_mm(0)
        kt[0] = kpool.tile([128, M], BF16, tag="kt", name="kt0")
        kproj_mm(0, kt[0], 0)
        kproj_mm(0, kt[0], 1)
        prev = None
        for hp in range(HP):
            if prev is not None:
                normalize(hp - 1, *prev)
            prev = attn_block(
                hp, build_v=(hp == 0), build_next=(hp < HP - 1)
            )
        normalize(HP - 1, *prev)

        # ---- output projection + bias: two 2-bank waves.  ci=5 runs last
        # as two packed K=64 matmuls so wave A needs only ot[5][0:64]
        # (ready after the first reciprocal); throwaway warm matmuls keep
        # the HAM clock at 8/8 while the last normalize drains. ----
        for wave in range(2):
            pss = []
            for n4 in (2 * wave, 2 * wave + 1):
                ps = ppool.tile([128, 1024], F32, tag="pp")
                pss.append(ps)
                for nh in range(2):
                    sl = slice(nh * 512, nh * 512 + 384)
                    for ci in range(CK - 1):
                        nc.tensor.matmul(
                            ps[:, sl],
                            ot[ci][:, n4 * 128:(n4 + 1) * 128],
                            wp[:, ci, nh * 384:(nh + 1) * 384],
                            start=(ci == 0),
                            stop=False,
                        )
            if wave == 0:
                wop = oppool.tile([128, 512], F32, tag="op", name="tailwarm")
                for i in range(16):
                    nc.tensor.matmul(
                        wop, wrm[:, 0:128], wrm,
                        start=(i == 0), stop=(i == 15),
                    )
            for i, n4 in enumerate((2 * wave, 2 * wave + 1)):
                ps = pss[i]
                for nh in range(2):
                    sl = slice(nh * 512, nh * 512 + 384)
                    nc.tensor.matmul(
                        ps[:, sl],
                        ot[CK - 1][:, n4 * 128:(n4 + 1) * 128],
                        wp[:, CK - 1, nh * 384:(nh + 1) * 384],
                        start=False,
                        stop=True,
                    )
                outs = outpool.tile([128, C], F32, tag="outs")
                for nh in range(2):
                    nc.vector.tensor_tensor(
                        outs[:, nh * 384:(nh + 1) * 384],
                        ps[:, nh * 512:nh * 512 + 384],
                        bpb[:, nh * 384:(nh + 1) * 384],
                        mybir.AluOpType.add,
                    )
                nc.sync.dma_start(out=out_c[n4], in_=outs)

    if not nc.is_finalized():
        nc.finalize()
    return nc


_NC_CACHE = None


def _get_nc():
    global _NC_CACHE
    if _NC_CACHE is None:
        _NC_CACHE = build_bass()
    return _NC_CACHE


def make_in_maps(x, y, yw, Wq, Wk, Wv, Wp, bp):
    import ml_dtypes

    bf = ml_dtypes.bfloat16
    x = np.asarray(x, np.float32)
    y = np.asarray(y, np.float32)
    yw = np.asarray(yw, np.float32)
    wqT = np.ascontiguousarray(np.asarray(Wq, np.float32).T).astype(bf)
    wkT = np.ascontiguousarray(np.asarray(Wk, np.float32).T).astype(bf)
    wvT = np.ascontiguousarray(np.asarray(Wv, np.float32).T).astype(bf)
    wpT = np.ascontiguousarray(np.asarray(Wp, np.float32).T).astype(bf)
    bpf = np.asarray(bp, np.float32).reshape(1, C)

    in_maps = []
    for c in range(N_CORES):
        b, half = divmod(c, 2)
        n0 = half * NSH
        in_maps.append(
            {
                "xT": np.ascontiguousarray(x[b, n0:n0 + NSH, :].T).astype(bf),
                "yT": np.ascontiguousarray(y[b].T).astype(bf),
                "ywr": np.ascontiguousarray(yw[b].reshape(1, M)).astype(bf),
                "wqT": wqT,
                "wkT": wkT,
                "wvT": wvT,
                "wpT": wpT,
                "bpf": bpf,
            }
        )
    return in_maps


def run(inputs, trace=False):
    """Returns (full_output, BassKernelResults)."""
    from concourse.bass_utils import run_bass_kernel_spmd

    nc = _get_nc()
    in_maps = make_in_maps(**inputs)
    res = run_bass_kernel_spmd(
        nc, in_maps, list(range(N_CORES)), trace=trace
    )
    full = np.empty((B, N, C), dtype=np.float32)
    for c in range(N_CORES):
        b, half = divmod(c, 2)
        n0 = half * NSH
        full[b, n0:n0 + NSH, :] = res.results[c]["out"]
    return full, res


def kernel(**inputs):
    full, _ = run(inputs, trace=False)
    return full


# revision 20
# speedup vs baseline: 2.1653x; 1.0289x over previous
"""Trainium2 Bass kernel for nn_CrossAttention (B=4, N=M=1024, C=768, H=12, D=64).

Sharding: pure data-parallel over 8 cores. Core c handles batch b = c // 2 and
query rows [512*(c%2), 512*(c%2)+512). Each core computes K/V for its batch
(duplicated across the 2 cores sharing a batch) so no collectives are needed.

All-bf16 datapath (fp32 PSUM accumulation); bf16 streams the PE at 1 cycle/row
and avoids the fp32r power throttle. Host-side layout:
  xT  [768, 512]   = x[b, n0:n0+512, :].T   (c-major for Q projection)
  yT  [768, 1024]  = y[b].T                 (c-major for K/V projection)
  w*T [768, 768]   = W.T                    (c-major weights)
  ywr [1, 1024]    = yw[b] row (bf16), bp fp32 row (DMA-replicated to 128
                     partitions for the DVE bias add)

Device dataflow (all matmuls bf16 x bf16 -> fp32 PSUM):
  QT[co,n] = sum_c wqT[c,co] xT[c,n]
  KT[co,m] = sum_c wkT[c,co] yT[c,m] + ones-row x ywr (rank-1 bias matmul)
  V[m,cv]  = sum_c yT[c,m] wvT[c,cv]  in a [128, 12, 128] per-chunk layout
             whose cols 64:128 are memset to 1 so the PV matmul's PSUM rows
             64:128 accumulate Z replicated 64x (softmax denominator, free)
  per head PAIR: two K=64 S-matmuls at PE array tile positions (0,0)/(64,0)
  (they execute concurrently on disjoint sub-arrays) into one [128,1024]
  PSUM tile, ONE exp over [128,1024] -> bf16, two PV matmuls.
  1/Z = stock DVE reciprocal on PSUM rows 64:128 (reciprocal_approx_fast is
  broken on this hardware), DVE multiply -> OT bf16.
  out[n,co] = sum_ci OT[ci,n] wpT[ci,co]; + bp via DVE add (replicated row).

Schedule notes (what the trace iterations taught us):
  - DMA: only use APs whose inner contiguous run is large. Each [128, 768]
    weight chunk of a (k p) n view is one linear 393KB region; the p-major
    single-DMA variant (768B segments) runs at ~2 GB/s/engine and starves
    everything.
  - ~8.5 us of throwaway warm-up matmuls at t=0 keep the HAM activity
    monitor at K=8/8 (2.4 GHz) while the weights load.
  - Block hp runs head-pair hp's 8-chunk attention with head-pair hp+1's
    Q/K projections interleaved at chunks 1/3/5 (V projection inside block
    0), so the PE never idles long enough to re-throttle.
  - Projection PSUM->SBUF casts run on the SCALAR engine: they land in the
    exp FIFO right where the PSUM pool rotation needs them, and keep the
    DVE free for the reciprocals (GpSimd cannot read PSUM; a DVE cast
    behind a 3.4 us reciprocal stalled the PE a full 7 us per block).
  - Output projection runs in two 2-bank PSUM waves, ci=5 last, so only
    the last pair's normalize sits on the critical path.
"""

import sys

for _p in ("/opt/trn_rl_repo",):
    if _p not in sys.path:
        sys.path.insert(0, _p)

import numpy as np
from contextlib import ExitStack

import concourse.bass as bass
import concourse.mybir as mybir
import concourse.tile as tile
from concourse import bacc

F32 = mybir.dt.float32
BF16 = mybir.dt.bfloat16

B = 4
N = 1024
M = 1024
C = 768
H = 12
D = 64
NSH = 512            # query rows per core
CK = C // 128        # 6 chunks of the feature dim
MK = M // 128        # 8 chunks of the key dim
HP = H // 2          # 6 head pairs (one KT/QT co-chunk each)
SCALE = D ** -0.5
N_CORES = 8
N_WARM = 28          # warm-up matmuls to keep HAM at 8/8 during loads


def build_bass():
    nc = bacc.Bacc("TRN2", target_bir_lowering=False, debug=False)

    xT = nc.dram_tensor("xT", [C, NSH], BF16, kind="ExternalInput").ap()
    yT = nc.dram_tensor("yT", [C, M], BF16, kind="ExternalInput").ap()
    ywr = nc.dram_tensor("ywr", [1, M], BF16, kind="ExternalInput").ap()
    wqT = nc.dram_tensor("wqT", [C, C], BF16, kind="ExternalInput").ap()
    wkT = nc.dram_tensor("wkT", [C, C], BF16, kind="ExternalInput").ap()
    wvT = nc.dram_tensor("wvT", [C, C], BF16, kind="ExternalInput").ap()
    wpT = nc.dram_tensor("wpT", [C, C], BF16, kind="ExternalInput").ap()
    bpf = nc.dram_tensor("bpf", [1, C], F32, kind="ExternalInput").ap()
    out = nc.dram_tensor("out", [NSH, C], F32, kind="ExternalOutput").ap()

    # k-major chunk views: each [128, x] chunk is one contiguous DRAM region
    wq_c = wqT.rearrange("(k p) n -> k p n", p=128)
    wk_c = wkT.rearrange("(k p) n -> k p n", p=128)
    wv_c = wvT.rearrange("(k p) n -> k p n", p=128)
    wp_c = wpT.rearrange("(k p) n -> k p n", p=128)
    xT_c = xT.rearrange("(k p) n -> k p n", p=128)
    yT_c = yT.rearrange("(k p) n -> k p n", p=128)
    out_c = out.rearrange("(k p) n -> k p n", p=128)

    with tile.TileContext(nc) as tc, ExitStack() as ctx:
        wpool = ctx.enter_context(tc.tile_pool(name="w", bufs=4))
        cpool = ctx.enter_context(tc.tile_pool(name="const", bufs=1))
        qpool = ctx.enter_context(tc.tile_pool(name="qt", bufs=3))
        kpool = ctx.enter_context(tc.tile_pool(name="kt", bufs=3))
        vpool = ctx.enter_context(tc.tile_pool(name="vs", bufs=MK))
        opool = ctx.enter_context(tc.tile_pool(name="ot", bufs=CK))
        epool = ctx.enter_context(tc.tile_pool(name="es", bufs=3))
        outpool = ctx.enter_context(tc.tile_pool(name="outs", bufs=2))
        zpool = ctx.enter_context(tc.tile_pool(name="z", bufs=4))
        ppool = ctx.enter_context(tc.tile_pool(name="pp", bufs=2, space="PSUM"))
        oppool = ctx.enter_context(tc.tile_pool(name="op", bufs=4, space="PSUM"))

        # ---- PE warm-up: throwaway matmuls with no DMA dependency ----
        wrm = cpool.tile([128, 512], BF16, tag="wrm")
        nc.gpsimd.memset(wrm, 0.0)
        ones = cpool.tile([1, C], BF16, tag="ones")
        nc.gpsimd.memset(ones, 1.0)
        wps = ppool.tile([128, 1024], F32, tag="pp", name="warmps")
        for i in range(N_WARM):
            nc.tensor.matmul(
                wps[:, 0:512], wrm[:, 0:128], wrm,
                start=(i == 0), stop=(i == N_WARM - 1),
            )

        # ---- input loads: contiguous per-chunk DMAs, 2 HWDGE rings ----
        # scalar ring: wq x6 (Q proj gate), yT x6, wp x6
        # sync ring:   xt x6, yw row, wk x6, wv x6, bp replicate
        wq = wpool.tile([128, CK, C], BF16, tag="w", name="wq")
        xt = cpool.tile([128, CK, NSH], BF16, tag="xt")
        for i in range(CK):
            nc.scalar.dma_start(out=wq[:, i, :], in_=wq_c[i])
            nc.sync.dma_start(out=xt[:, i, :], in_=xT_c[i])
        yw_s = cpool.tile([1, M], BF16, tag="yws")
        nc.sync.dma_start(out=yw_s, in_=ywr)
        yt = cpool.tile([128, CK, M], BF16, tag="yt")
        wk = wpool.tile([128, CK, C], BF16, tag="w", name="wk")
        for i in range(CK):
            nc.scalar.dma_start(out=yt[:, i, :], in_=yT_c[i])
            nc.sync.dma_start(out=wk[:, i, :], in_=wk_c[i])
        wv = wpool.tile([128, CK, C], BF16, tag="w", name="wv")
        for i in range(CK):
            nc.sync.dma_start(out=wv[:, i, :], in_=wv_c[i])
        # wp is not needed until the output projection: load it via the
        # gpsimd SWDGE ring so it costs neither HWDGE ring any bandwidth
        # and adds no issue time on the ACT queue.
        wp = wpool.tile([128, CK, C], BF16, tag="w", name="wp")
        for i in range(CK):
            nc.gpsimd.dma_start(out=wp[:, i, :], in_=wp_c[i])
        bpb = cpool.tile([128, C], F32, tag="bpb")
        nc.sync.dma_start(
            out=bpb,
            in_=bass.AP(tensor=bpf.tensor, offset=0, ap=[[0, 128], [1, C]]),
        )
        # preload the ACT exp table from the memset ones tile: zero DMA
        # dependency, so the table load cannot block the projection casts
        # queued behind it on the ACT engine.
        warm = cpool.tile([1, 8], BF16, tag="warm")
        nc.scalar.activation(
            warm, ones[0:1, 0:8], mybir.ActivationFunctionType.Exp,
            scale=SCALE,
        )

        def qproj_mm(co):
            ps = ppool.tile([128, 1024], F32, tag="pp")
            for ci in range(CK):
                nc.tensor.matmul(
                    ps[:, 0:512],
                    wq[:, ci, co * 128:(co + 1) * 128],
                    xt[:, ci, :],
                    start=(ci == 0),
                    stop=(ci == CK - 1),
                )
            t = qpool.tile([128, NSH], BF16, tag="qt")
            nc.scalar.copy(t, ps[:, 0:512])
            return t

        def kproj_mm(co, t, mh):
            ps = ppool.tile([128, 1024], F32, tag="pp")
            sl = slice(mh * 512, (mh + 1) * 512)
            for ci in range(CK):
                nc.tensor.matmul(
                    ps[:, 0:512],
                    wk[:, ci, co * 128:(co + 1) * 128],
                    yt[:, ci, sl],
                    start=(ci == 0),
                    stop=False,
                )
            # += ones-row^T x yw-row: the additive key bias, rank-1
            nc.tensor.matmul(
                ps[:, 0:512],
                ones[:, co * 128:(co + 1) * 128],
                yw_s[:, sl],
                start=False,
                stop=True,
            )
            nc.scalar.copy(t[:, sl], ps[:, 0:512])

        def vproj(mc):
            t = vpool.tile([128, H, 128], BF16, tag="vs")
            nc.gpsimd.memset(t[:, :, 64:128], 1.0)
            ps = ppool.tile([128, 1024], F32, tag="pp")
            for nh in range(2):
                sl = slice(nh * 512, nh * 512 + 384)
                for ci in range(CK):
                    nc.tensor.matmul(
                        ps[:, sl],
                        yt[:, ci, mc * 128:(mc + 1) * 128],
                        wv[:, ci, nh * 384:(nh + 1) * 384],
                        start=(ci == 0),
                        stop=(ci == CK - 1),
                    )
            for nh in range(2):
                src = ps[:, nh * 512:nh * 512 + 384].rearrange(
                    "p (h e) -> p h e", e=64
                )
                nc.vector.tensor_copy(t[:, nh * 6:(nh + 1) * 6, 0:64], src)
            return t

        vt = [None] * MK
        ot = [None] * HP
        qt = [None] * HP
        kt = [None] * HP

        def attn_block(hp, build_v, build_next):
            """Head-pair hp's attention; next pair's projections (and, for
            hp==0, the V projection) sliced into the chunk loop.  The PV
            pair for chunk mc is emitted during chunk mc+1 so the PE never
            sits behind the in-flight exp."""
            h0, h1 = 2 * hp, 2 * hp + 1
            qtile, ktile = qt[hp], kt[hp]
            op0 = oppool.tile([128, 512], F32, tag="op", name=f"op{h0}")
            op1 = oppool.tile([128, 512], F32, tag="op", name=f"op{h1}")
            nxt = hp + 1
            ess = [None] * MK

            def pv(mc):
                nc.tensor.matmul(
                    op0, vt[mc][:, h0, :], ess[mc][:, 0:512],
                    start=(mc == 0), stop=(mc == MK - 1),
                )
                nc.tensor.matmul(
                    op1, vt[mc][:, h1, :], ess[mc][:, 512:1024],
                    start=(mc == 0), stop=(mc == MK - 1),
                )

            def proj_slice(slot):
                if not build_next:
                    return
                # each branch allocates ONE psum tile; the extra untouched
                # dummy alloc keeps the 2-buffer pp rotation parity so
                # S-pair(mc) reuses sp(mc-2) (pipeline depth 2), not the
                # tile freed by the previous chunk's exp.
                if slot == 1:
                    qt[nxt] = qproj_mm(nxt)
                    ppool.tile([128, 1024], F32, tag="pp", name=f"dq{nxt}")
                elif slot == 3:
                    kt[nxt] = kpool.tile(
                        [128, M], BF16, tag="kt", name=f"kt{nxt}"
                    )
                    kproj_mm(nxt, kt[nxt], 0)
                    ppool.tile([128, 1024], F32, tag="pp", name=f"da{nxt}")
                elif slot == 5:
                    kproj_mm(nxt, kt[nxt], 1)
                    ppool.tile([128, 1024], F32, tag="pp", name=f"db{nxt}")

            def s_pair(mc):
                sp = ppool.tile([128, 1024], F32, tag="pp")
                nc.tensor.matmul(
                    sp[:, 0:512],
                    ktile[0:64, mc * 128:(mc + 1) * 128],
                    qtile[0:64, :],
                    start=True,
                    stop=True,
                )
                nc.tensor.matmul(
                    sp[:, 512:1024],
                    ktile[64:128, mc * 128:(mc + 1) * 128],
                    qtile[64:128, :],
                    start=True,
                    stop=True,
                )
                es = epool.tile([128, 1024], BF16, tag="es")
                nc.scalar.activation(
                    es, sp, mybir.ActivationFunctionType.Exp, scale=SCALE
                )
                ess[mc] = es

            # S-pair for chunk mc+1 is emitted BEFORE chunk mc's PV and
            # projection work, so those fill the PE's exp-wait shadow
            # instead of delaying the next score matmul (and the exp fed
            # from it).
            s_pair(0)
            for mc in range(MK):
                if mc + 1 < MK:
                    s_pair(mc + 1)
                if build_v:
                    vt[mc] = vproj(mc)
                if mc >= 1:
                    pv(mc - 1)
                proj_slice(mc)
            pv(MK - 1)
            return op0, op1

        def normalize(hp, op0, op1):
            t = opool.tile([128, NSH], BF16, tag="ot", name=f"ot{hp}")
            for j, op in enumerate((op0, op1)):
                zr = zpool.tile([64, 512], F32, tag="z")
                nc.vector.reciprocal(zr, op[64:128, :])
                nc.vector.tensor_tensor(
                    t[j * 64:(j + 1) * 64, :], op[0:64, :], zr,
                    mybir.AluOpType.mult,
                )
            ot[hp] = t

        qt[0] = qproj_mm(0)
        kt[0] = kpool.tile([128, M], BF16, tag="kt", name="kt0")
        kproj_mm(0, kt[0], 0)
        kproj_mm(0, kt[0], 1)
        prev = None
        for hp in range(HP):
            if prev is not None:
                normalize(hp - 1, *prev)
            prev = attn_block(
                hp, build_v=(hp == 0), build_next=(hp < HP - 1)
            )
        normalize(HP - 1, *prev)

        # ---- output projection + bias: two 2-bank waves.  ci=5 runs last
        # as two packed K=64 matmuls so wave A needs only ot[5][0:64]
        # (ready after the first reciprocal); throwaway warm matmuls keep
        # the HAM clock at 8/8 while the last normalize drains. ----
        for wave in range(2):
            pss = []
            for n4 in (2 * wave, 2 * wave + 1):
                ps = ppool.tile([128, 1024], F32, tag="pp")
                pss.append(ps)
                for nh in range(2):
                    sl = slice(nh * 512, nh * 512 + 384)
                    for ci in range(CK - 1):
                        nc.tensor.matmul(
                            ps[:, sl],
                            ot[ci][:, n4 * 128:(n4 + 1) * 128],
                            wp[:, ci, nh * 384:(nh + 1) * 384],
                            start=(ci == 0),
                            stop=False,
                        )
            if wave == 0:
                wop = oppool.tile([128, 512], F32, tag="op", name="tailwarm")
                for i in range(16):
                    nc.tensor.matmul(
                        wop, wrm[:, 0:128], wrm,
                        start=(i == 0), stop=(i == 15),
                    )
            for i, n4 in enumerate((2 * wave, 2 * wave + 1)):
                ps = pss[i]
                for nh in range(2):
                    sl = slice(nh * 512, nh * 512 + 384)
                    nc.tensor.matmul(
                        ps[:, sl],
                        ot[CK - 1][:, n4 * 128:(n4 + 1) * 128],
                        wp[:, CK - 1, nh * 384:(nh + 1) * 384],
                        start=False,
                        stop=True,
                    )
                outs = outpool.tile([128, C], F32, tag="outs")
                for nh in range(2):
                    nc.vector.tensor_tensor(
                        outs[:, nh * 384:(nh + 1) * 384],
                        ps[:, nh * 512:nh * 512 + 384],
                        bpb[:, nh * 384:(nh + 1) * 384],
                        mybir.AluOpType.add,
                    )
                nc.sync.dma_start(out=out_c[n4], in_=outs)

    if not nc.is_finalized():
        nc.finalize()
    return nc


_NC_CACHE = None


def _get_nc():
    global _NC_CACHE
    if _NC_CACHE is None:
        _NC_CACHE = build_bass()
    return _NC_CACHE


def make_in_maps(x, y, yw, Wq, Wk, Wv, Wp, bp):
    import ml_dtypes

    bf = ml_dtypes.bfloat16
    x = np.asarray(x, np.float32)
    y = np.asarray(y, np.float32)
    yw = np.asarray(yw, np.float32)
    wqT = np.ascontiguousarray(np.asarray(Wq, np.float32).T).astype(bf)
    wkT = np.ascontiguousarray(np.asarray(Wk, np.float32).T).astype(bf)
    wvT = np.ascontiguousarray(np.asarray(Wv, np.float32).T).astype(bf)
    wpT = np.ascontiguousarray(np.asarray(Wp, np.float32).T).astype(bf)
    bpf = np.asarray(bp, np.float32).reshape(1, C)

    in_maps = []
    for c in range(N_CORES):
        b, half = divmod(c, 2)
        n0 = half * NSH
        in_maps.append(
            {
                "xT": np.ascontiguousarray(x[b, n0:n0 + NSH, :].T).astype(bf),
                "yT": np.ascontiguousarray(y[b].T).astype(bf),
                "ywr": np.ascontiguousarray(yw[b].reshape(1, M)).astype(bf),
                "wqT": wqT,
                "wkT": wkT,
                "wvT": wvT,
                "wpT": wpT,
                "bpf": bpf,
            }
        )
    return in_maps


def run(inputs, trace=False):
    """Returns (full_output, BassKernelResults)."""
    from concourse.bass_utils import run_bass_kernel_spmd

    nc = _get_nc()
    in_maps = make_in_maps(**inputs)
    res = run_bass_kernel_spmd(
        nc, in_maps, list(range(N_CORES)), trace=trace
    )
    full = np.empty((B, N, C), dtype=np.float32)
    for c in range(N_CORES):
        b, half = divmod(c, 2)
        n0 = half * NSH
        full[b, n0:n0 + NSH, :] = res.results[c]["out"]
    return full, res


def kernel(**inputs):
    full, _ = run(inputs, trace=False)
    return full
